# revision 1
# baseline (speedup 1.0000x reference)
"""Trainium2 Bass kernel for nn_CaptioningRNN (attention LSTM over T=64).

Data-parallel over the batch: N=256 samples split across 8 NeuronCores
(32 samples/core), weights replicated, no collectives.

Per-core algorithm (all matmuls bf16 on the TensorEngine, state in f32):
  1. xproj phase: xpT = (x @ Wx + b) computed transposed via Wx-stationary
     matmuls, stored to a DRAM scratch laid out so the per-step slice loads
     as a clean [128, 512] tile.
  2. P phase: P[n, k, :] = A[n, :, k] @ Wattn precomputed once (the
     attention context contribution to the gates becomes a w-weighted sum
     of P rows, replacing a per-step [32,1024]@[1024,4096] matmul).
     h0 = c0 = mean_k(A) computed on device from an f32 copy of A.
  3. Recurrence (64 steps):
     - scores via hT-chunk matmuls against a permuted A (cross-sample
       products in PSUM, diagonal extracted with a mask+reduce on DVE)
     - softmax on [32,16] (ACT exp with fused row-sum)
     - w transposed (DVE 32x32 stream transpose) and expanded to the
       (k, n_g)-partition block-diagonal layout via a one-hot matmul + mask
     - gates = h @ Wh + sum_k w_k P_k accumulated into two PSUM strips
       using 4-way tensor-engine column tiling (beats the M=32 small-batch
       penalty ~4x)
     - strips transposed on PE; cell math done in h-on-partition space so
       i/f/o/g land on identical lanes (no cross-partition ops needed)
  4. Output written transposed [t, h, n]; host reassembles to (N, T, H).
"""

import numpy as np
import ml_dtypes

import concourse.bacc as bacc
import concourse.mybir as mybir
from concourse import bass_utils
from concourse.tile import TileContext

F32, BF16 = mybir.dt.float32, mybir.dt.bfloat16
AF = mybir.ActivationFunctionType
ALU = mybir.AluOpType
AX = mybir.AxisListType
BF = ml_dtypes.bfloat16

N, T, D, H = 256, 64, 1024, 1024
NCORES = 8
NL = N // NCORES          # 32 samples per core
HC = 8                    # 128-row chunks of D/H
G, GS = 4, 8              # sample groups of 8 (for the (k, n_g) 128-partition layout)
H4 = 4 * H                # 4096 gate columns

_built = None


def _consts():
    # E16[k', 8k + n] = (k' == k): one-hot expansion of wT rows onto the
    # (k-major, n_g-minor) 128-partition layout.
    e16 = np.zeros((16, 128), dtype=BF)
    for k in range(16):
        e16[k, 8 * k : 8 * k + 8] = 1
    # M32R[p, 128 g + 32 rep + m] = (m % 8 == p % 8) & (m // 8 == g):
    # block-diagonal mask producing masked_g = w[m, k(p)] only for group-g
    # samples, replicated 4x for the column-tiled matmuls.
    p = np.arange(128)[:, None]
    m = np.arange(32)[None, :]
    m32r = np.zeros((128, 512), dtype=BF)
    for g in range(4):
        blk = ((m % 8 == p % 8) & (m // 8 == g)).astype(BF)
        for rep in range(4):
            m32r[:, 128 * g + 32 * rep : 128 * g + 32 * rep + 32] = blk
    # Mdiag[n, 32 k + n'] = (n == n') / 32: extracts the diagonal of the
    # cross-sample score products and applies the 1/sqrt(H) softmax scale.
    md = np.zeros((32, 512), dtype=np.float32)
    n_ = np.arange(32)
    for k in range(16):
        md[n_, 32 * k + n_] = 1.0 / 32.0
    return e16, m32r, md


def _build_nc(t_steps=T):
    nc = bacc.Bacc(trn_type="TRN2", target_bir_lowering=False, debug=False)

    ap_xT = nc.dram_tensor("xT", [D, T * NL], BF16, kind="ExternalInput").ap()
    ap_Asc = nc.dram_tensor("Asc", [H, 512], BF16, kind="ExternalInput").ap()
    ap_Asc32 = nc.dram_tensor("Asc32", [H, 512], F32, kind="ExternalInput").ap()
    ap_Wx = nc.dram_tensor("Wx", [D, H4], BF16, kind="ExternalInput").ap()
    ap_Wh = nc.dram_tensor("Wh", [H, H4], BF16, kind="ExternalInput").ap()
    ap_Wattn = nc.dram_tensor("Wattn", [H, H4], BF16, kind="ExternalInput").ap()
    ap_bT = nc.dram_tensor("bT", [128, 32], F32, kind="ExternalInput").ap()
    outT = nc.dram_tensor("outT", [T, H, NL], F32, kind="ExternalOutput").ap()
    # xps[t, r, p, q, j, n] = xproj[t][n, j*1024 + r*512 + q*128 + p]
    xps = nc.dram_tensor("xps", [T, 2, 128, 4, 4, NL], F32, kind="Internal").ap()

    e16_np, m32r_np, md_np = _consts()
    eye_d = nc.inline_tensor(np.eye(128, dtype=np.float32), "c_eye")
    e16_d = nc.inline_tensor(e16_np, "c_e16")
    m32r_d = nc.inline_tensor(m32r_np, "c_m32r")
    md_d = nc.inline_tensor(md_np, "c_mdiag")

    with TileContext(nc) as tc:
        with tc.tile_pool(name="pers", bufs=1) as pers:
            Wh_sb = pers.tile([128, HC * H4], BF16, tag="Wh")
            Asc_sb = pers.tile([128, HC * 512], BF16, tag="Asc")
            P_sb = pers.tile([128, G * H4], BF16, tag="P")
            uTh = pers.tile([128, HC * 128], BF16, tag="uTh")
            cT = pers.tile([128, 256], F32, tag="cT")
            eye = pers.tile([128, 128], F32, tag="eye")
            E16 = pers.tile([16, 128], BF16, tag="E16")
            M32R = pers.tile([128, 512], BF16, tag="M32R")
            Mdiag = pers.tile([32, 512], F32, tag="Mdiag")
            b_sb = pers.tile([128, 32], F32, tag="bT")
            wsq = pers.tile([32, 32], F32, tag="wsq")

            nc.sync.dma_start(eye[:], eye_d.ap()[:])
            nc.sync.dma_start(E16[:], e16_d.ap()[:])
            nc.sync.dma_start(M32R[:], m32r_d.ap()[:])
            nc.sync.dma_start(Mdiag[:], md_d.ap()[:])
            nc.sync.dma_start(b_sb[:], ap_bT[:])
            nc.gpsimd.memset(wsq[:], 0.0)
            for c in range(HC):
                nc.sync.dma_start(
                    Wh_sb[:, c * H4 : (c + 1) * H4], ap_Wh[128 * c : 128 * (c + 1), :]
                )
                nc.sync.dma_start(
                    Asc_sb[:, c * 512 : (c + 1) * 512],
                    ap_Asc[128 * c : 128 * (c + 1), :],
                )

            # ---------------- phase A: xproj -> DRAM scratch ----------------
            with tc.tile_pool(name="phx1", bufs=1) as phx1, \
                 tc.tile_pool(name="phx", bufs=3) as phx, \
                 tc.tile_pool(name="psX", bufs=2, space="PSUM") as psX:
                xT_sb = phx1.tile([128, HC * T * NL], BF16, tag="xTsb")
                for c in range(HC):
                    nc.sync.dma_start(
                        xT_sb[:, c * T * NL : (c + 1) * T * NL],
                        ap_xT[128 * c : 128 * (c + 1), :],
                    )
                for W in range(32):
                    j, r, q = W // 8, (W % 8) // 4, W % 4
                    Wxb = phx.tile([128, HC * 128], BF16, tag="Wxb")
                    for c in range(HC):
                        nc.sync.dma_start(
                            Wxb[:, c * 128 : (c + 1) * 128],
                            ap_Wx[128 * c : 128 * (c + 1), 128 * W : 128 * (W + 1)],
                        )
                    for t4 in range(4):
                        psx = psX.tile([128, 512], F32, tag="psx")
                        for c in range(HC):
                            nc.tensor.matmul(
                                psx[:],
                                Wxb[:, c * 128 : (c + 1) * 128],
                                xT_sb[:, c * T * NL + 512 * t4 : c * T * NL + 512 * (t4 + 1)],
                                start=(c == 0),
                                stop=(c == HC - 1),
                            )
                        sxp = phx.tile([128, 512], F32, tag="sxp")
                        nc.vector.tensor_scalar_add(sxp[:], psx[:], b_sb[:, W : W + 1])
                        nc.sync.dma_start(
                            xps[16 * t4 : 16 * (t4 + 1), r, :, q, j, :].transpose(
                                [1, 0, 2]
                            ),
                            sxp[:].rearrange("p (t n) -> p t n", t=16),
                        )

            # ------------- phase B: P precompute + h0/c0 init -------------
            with tc.tile_pool(name="php1", bufs=1) as php1, \
                 tc.tile_pool(name="php", bufs=3) as php, \
                 tc.tile_pool(name="psP", bufs=2, space="PSUM") as psP:
                A32 = php1.tile([128, HC * 512], F32, tag="A32")
                for c in range(HC):
                    nc.sync.dma_start(
                        A32[:, c * 512 : (c + 1) * 512],
                        ap_Asc32[128 * c : 128 * (c + 1), :],
                    )
                for c in range(HC):
                    h0s = php.tile([128, 32], F32, tag="h0s")
                    nc.vector.tensor_reduce(
                        h0s[:],
                        A32[:, c * 512 : (c + 1) * 512].rearrange(
                            "p (k n) -> p n k", k=16
                        ),
                        axis=AX.X,
                        op=ALU.add,
                    )
                    nc.vector.tensor_scalar_mul(
                        cT[:, 32 * c : 32 * (c + 1)], h0s[:], 1.0 / 16.0
                    )
                    for rep in range(4):
                        nc.vector.tensor_copy(
                            uTh[:, 128 * c + 32 * rep : 128 * c + 32 * (rep + 1)],
                            cT[:, 32 * c : 32 * (c + 1)],
                        )
                # contiguous staging of the group-selected A columns so the
                # matmul stationary operand has a single free dim
                Ag = php1.tile([128, G * HC * 128], BF16, tag="Ag")
                for g in range(G):
                    for c in range(HC):
                        nc.vector.tensor_copy(
                            Ag[:, (g * HC + c) * 128 : (g * HC + c) * 128 + 128],
                            Asc_sb[:, c * 512 : (c + 1) * 512].rearrange(
                                "p (k n) -> p k n", k=16
                            )[:, :, GS * g : GS * (g + 1)],
                        )
                for blk in range(8):
                    Wab = php.tile([128, HC * 512], BF16, tag="Wab")
                    for c in range(HC):
                        nc.sync.dma_start(
                            Wab[:, c * 512 : (c + 1) * 512],
                            ap_Wattn[128 * c : 128 * (c + 1), 512 * blk : 512 * (blk + 1)],
                        )
                    for g in range(G):
                        psp = psP.tile([128, 512], F32, tag="psp")
                        for c in range(HC):
                            nc.tensor.matmul(
                                psp[:],
                                Ag[:, (g * HC + c) * 128 : (g * HC + c) * 128 + 128],
                                Wab[:, c * 512 : (c + 1) * 512],
                                start=(c == 0),
                                stop=(c == HC - 1),
                            )
                        nc.vector.tensor_copy(
                            P_sb[:, g * H4 + 512 * blk : g * H4 + 512 * (blk + 1)],
                            psp[:],
                        )

            # ---------------------- phase C: recurrence ----------------------
            with tc.tile_pool(name="wrk", bufs=2) as wrk, \
                 tc.tile_pool(name="psc", bufs=2, space="PSUM") as psc_pool, \
                 tc.tile_pool(name="pwx", bufs=1, space="PSUM") as pwx_pool, \
                 tc.tile_pool(name="pstr", bufs=1, space="PSUM") as pstr_pool, \
                 tc.tile_pool(name="paT", bufs=1, space="PSUM") as paT_pool:
                q4 = lambda ap: ap.rearrange("p (q c) -> p q c", q=4)
                for t in range(t_steps):
                    # prefetched xproj slices for this step
                    xpt = [wrk.tile([128, 512], F32, tag=f"xpt{r}", name=f"xpt{r}_{t}") for r in range(2)]
                    for r in range(2):
                        nc.sync.dma_start(xpt[r][:], xps[t, r])

                    # -- scores: cross-sample products, diag extract, softmax
                    psc = psc_pool.tile([32, 512], F32, tag="psc")
                    for c in range(HC):
                        nc.tensor.matmul(
                            psc[:],
                            uTh[:, c * 128 : c * 128 + 32],
                            Asc_sb[:, c * 512 : (c + 1) * 512],
                            start=(c == 0),
                            stop=(c == HC - 1),
                        )
                    scm = wrk.tile([32, 512], F32, tag="scm")
                    nc.vector.tensor_mul(scm[:], psc[:], Mdiag[:])
                    scores = wrk.tile([32, 16], F32, tag="scores")
                    nc.vector.tensor_reduce(
                        scores[:],
                        scm[:].rearrange("p (k n) -> p k n", k=16),
                        axis=AX.X,
                        op=ALU.add,
                    )
                    nmx = wrk.tile([32, 1], F32, tag="nmx")
                    nc.vector.tensor_reduce(
                        nmx[:], scores[:], axis=AX.X, op=ALU.max, negate=True
                    )
                    ex = wrk.tile([32, 16], F32, tag="ex")
                    esum = wrk.tile([32, 1], F32, tag="esum")
                    nc.scalar.activation(
                        ex[:], scores[:], AF.Exp, bias=nmx[:], scale=1.0,
                        accum_out=esum[:],
                    )
                    rcp = wrk.tile([32, 1], F32, tag="rcp")
                    nc.vector.reciprocal(rcp[:], esum[:])
                    nc.vector.tensor_scalar_mul(wsq[:, 0:16], ex[:], rcp[:])
                    wT = wrk.tile([32, 32], F32, tag="wT")
                    nc.vector.transpose(wT[:], wsq[:])
                    wrep = wrk.tile([16, 128], BF16, tag="wrep")
                    for rep in range(4):
                        nc.vector.tensor_copy(
                            wrep[:, 32 * rep : 32 * (rep + 1)], wT[0:16, 0:32]
                        )
                    pwx = pwx_pool.tile([128, 128], F32, tag="pwx")
                    nc.tensor.matmul(pwx[:], E16[:], wrep[:], start=True, stop=True)
                    masked = wrk.tile([128, 512], BF16, tag="masked")
                    for g in range(G):
                        nc.vector.tensor_mul(
                            masked[:, g * 128 : (g + 1) * 128],
                            pwx[:],
                            M32R[:, g * 128 : (g + 1) * 128],
                        )

                    # -- gates: h @ Wh + sum_k w_k P_k into 2 column-tiled strips
                    strips = [
                        pstr_pool.tile([128, 512], F32, tag=f"strip{r}",
                                       name=f"strip{r}_{t}")
                        for r in range(2)
                    ]
                    for c in range(HC):
                        for r in range(2):
                            for j in range(4):
                                nc.tensor.matmul(
                                    strips[r][32 * j : 32 * (j + 1), :],
                                    uTh[:, c * 128 + 32 * j : c * 128 + 32 * (j + 1)],
                                    Wh_sb[:, c * H4 + j * 1024 + r * 512 : c * H4 + j * 1024 + r * 512 + 512],
                                    start=(c == 0),
                                    stop=False,
                                    skip_group_check=True,
                                    tile_position=(0, 32 * j),
                                )
                    for r in range(2):
                        for g in range(G):
                            for j in range(4):
                                nc.tensor.matmul(
                                    strips[r][32 * j : 32 * (j + 1), :],
                                    masked[:, g * 128 + 32 * j : g * 128 + 32 * (j + 1)],
                                    P_sb[:, g * H4 + j * 1024 + r * 512 : g * H4 + j * 1024 + r * 512 + 512],
                                    start=False,
                                    stop=(g == G - 1),
                                    skip_group_check=True,
                                    tile_position=(0, 32 * j),
                                )

                    # -- transpose strips, add xproj, activations, cell update
                    for r in range(2):
                        sg = wrk.tile([128, 512], F32, tag=f"sg{r}")
                        nc.vector.tensor_copy(sg[:], strips[r][:])
                        pat = paT_pool.tile([128, 512], F32, tag=f"pat{r}")
                        for q in range(4):
                            nc.tensor.matmul(
                                pat[:, 128 * q : 128 * (q + 1)],
                                sg[:, 128 * q : 128 * (q + 1)],
                                eye[:],
                                is_transpose=True,
                                start=(q == 0),
                                stop=(q == 3),
                            )
                        ssum = wrk.tile([128, 512], F32, tag=f"ssum{r}")
                        nc.vector.tensor_add(ssum[:], pat[:], xpt[r][:])
                        act = wrk.tile([128, 512], F32, tag=f"act{r}")
                        nc.scalar.activation(
                            q4(act[:])[:, :, 0:96], q4(ssum[:])[:, :, 0:96], AF.Sigmoid
                        )
                        nc.scalar.activation(
                            q4(act[:])[:, :, 96:128], q4(ssum[:])[:, :, 96:128], AF.Tanh
                        )
                        i_v = q4(act[:])[:, :, 0:32]
                        f_v = q4(act[:])[:, :, 32:64]
                        o_v = q4(act[:])[:, :, 64:96]
                        g_v = q4(act[:])[:, :, 96:128]
                        cview = cT[:, 128 * r : 128 * (r + 1)].rearrange(
                            "p (q n) -> p q n", q=4
                        )
                        ig = wrk.tile([128, 128], F32, tag=f"ig{r}")
                        nc.vector.tensor_mul(q4(ig[:]), i_v, g_v)
                        fc = wrk.tile([128, 128], F32, tag=f"fc{r}")
                        nc.vector.tensor_mul(q4(fc[:]), f_v, cview)
                        nc.vector.tensor_add(
                            cview, q4(ig[:]), q4(fc[:])
                        )
                        tch = wrk.tile([128, 128], F32, tag=f"tch{r}")
                        nc.scalar.activation(
                            tch[:], cT[:, 128 * r : 128 * (r + 1)], AF.Tanh
                        )
                        h32 = wrk.tile([128, 128], F32, tag=f"h32{r}")
                        nc.vector.tensor_mul(
                            h32[:].rearrange("p (q n) -> p q n", q=4),
                            o_v,
                            tch[:].rearrange("p (q n) -> p q n", q=4),
                        )
                        # write h into uTh (bf16, 4 replicas) for step t+1
                        uv = uTh[:].rearrange("p (c rep n) -> p c rep n", c=HC, rep=4)
                        for rep in range(4):
                            nc.vector.tensor_copy(
                                uv[:, 4 * r : 4 * (r + 1), rep, :],
                                h32[:].rearrange("p (q n) -> p q n", q=4),
                            )
                        nc.sync.dma_start(
                            outT[t, 512 * r : 512 * (r + 1), :].rearrange(
                                "(q p) n -> p q n", p=128
                            ),
                            h32[:].rearrange("p (q n) -> p q n", q=4),
                        )
    nc.compile()
    return nc


def _prep_shards(inputs):
    x = np.asarray(inputs["x"], np.float32)
    A = np.asarray(inputs["A"], np.float32)
    Wx = np.asarray(inputs["Wx"], np.float32)
    Wh = np.asarray(inputs["Wh"], np.float32)
    Wattn = np.asarray(inputs["Wattn"], np.float32)
    b = np.asarray(inputs["b"], np.float32)

    Wx_bf = np.ascontiguousarray(Wx.astype(BF))
    Wh_bf = np.ascontiguousarray(Wh.astype(BF))
    Wa_bf = np.ascontiguousarray(Wattn.astype(BF))
    bT = np.ascontiguousarray(b.reshape(32, 128).T.astype(np.float32))

    in_maps = []
    for i in range(NCORES):
        ns = slice(NL * i, NL * (i + 1))
        xT = x[ns].transpose(2, 1, 0).reshape(D, T * NL)
        Asc = A[ns].reshape(NL, H, 16).transpose(1, 2, 0).reshape(H, 512)
        in_maps.append(
            {
                "xT": np.ascontiguousarray(xT.astype(BF)),
                "Asc": np.ascontiguousarray(Asc.astype(BF)),
                "Asc32": np.ascontiguousarray(Asc.astype(np.float32)),
                "Wx": Wx_bf,
                "Wh": Wh_bf,
                "Wattn": Wa_bf,
                "bT": bT,
            }
        )
    return in_maps


def _get_nc():
    global _built
    if _built is None:
        _built = _build_nc()
    return _built


def _run(inputs, **kwargs):
    nc = _get_nc()
    in_maps = _prep_shards(inputs)
    res = bass_utils.run_bass_kernel_spmd(
        nc, in_maps, core_ids=list(range(NCORES)), **kwargs
    )
    out = np.empty((N, T, H), np.float32)
    for i in range(NCORES):
        out[NL * i : NL * (i + 1)] = res.results[i]["outT"].transpose(2, 0, 1)
    return out, res


def kernel(**inputs):
    out, _ = _run(inputs)
    return out



# revision 12
# speedup vs baseline: 1.0452x; 1.0452x over previous
"""Trainium2 Bass kernel for nn_CaptioningRNN (attention LSTM over T=64).

Data-parallel over the batch: N=256 samples split across 8 NeuronCores
(32 samples/core), weights replicated, no collectives.

Per-core algorithm (all matmuls bf16 on the TensorEngine, state in f32):
  1. xproj phase: xpT = (x @ Wx + b) computed transposed via Wx-stationary
     matmuls, stored to a DRAM scratch laid out so the per-step slice loads
     as a clean [128, 512] tile.
  2. P phase: P[n, k, :] = A[n, :, k] @ Wattn precomputed once (the
     attention context contribution to the gates becomes a w-weighted sum
     of P rows, replacing a per-step [32,1024]@[1024,4096] matmul).
     h0 = c0 = mean_k(A) computed on device from an f32 copy of A.
  3. Recurrence (64 steps):
     - scores via hT-chunk matmuls against a permuted A (cross-sample
       products in PSUM, diagonal extracted with a mask+reduce on DVE)
     - softmax on [32,16] (ACT exp with fused row-sum)
     - w transposed (DVE 32x32 stream transpose) and expanded to the
       (k, n_g)-partition block-diagonal layout via a one-hot matmul + mask
     - gates = h @ Wh + sum_k w_k P_k accumulated into two PSUM strips
       using 4-way tensor-engine column tiling (beats the M=32 small-batch
       penalty ~4x)
     - strips transposed on PE; cell math done in h-on-partition space so
       i/f/o/g land on identical lanes (no cross-partition ops needed)
  4. Output written transposed [t, h, n]; host reassembles to (N, T, H).
"""

import numpy as np
import ml_dtypes

import concourse.bacc as bacc
import concourse.mybir as mybir
from concourse import bass_utils
from concourse.tile import TileContext

F32, BF16 = mybir.dt.float32, mybir.dt.bfloat16
AF = mybir.ActivationFunctionType
ALU = mybir.AluOpType
AX = mybir.AxisListType
BF = ml_dtypes.bfloat16

N, T, D, H = 256, 64, 1024, 1024
NCORES = 8
NL = N // NCORES          # 32 samples per core
HC = 8                    # 128-row chunks of D/H
G, GS = 4, 8              # sample groups of 8 (for the (k, n_g) 128-partition layout)
H4 = 4 * H                # 4096 gate columns

_built = None


def _consts():
    # E16[k', 8k + n] = (k' == k): one-hot expansion of wT rows onto the
    # (k-major, n_g-minor) 128-partition layout.
    e16 = np.zeros((16, 128), dtype=BF)
    for k in range(16):
        e16[k, 8 * k : 8 * k + 8] = 1
    # M32R[p, 128 g + 32 rep + m] = (m % 8 == p % 8) & (m // 8 == g):
    # block-diagonal mask producing masked_g = w[m, k(p)] only for group-g
    # samples, replicated 4x for the column-tiled matmuls.
    p = np.arange(128)[:, None]
    m = np.arange(32)[None, :]
    m32r = np.zeros((128, 512), dtype=BF)
    for g in range(4):
        blk = ((m % 8 == p % 8) & (m // 8 == g)).astype(BF)
        for rep in range(4):
            m32r[:, 128 * g + 32 * rep : 128 * g + 32 * rep + 32] = blk
    # Mdiag[n, 32 k + n'] = (n == n') / 32: extracts the diagonal of the
    # cross-sample score products and applies the 1/sqrt(H) softmax scale.
    md = np.zeros((32, 512), dtype=np.float32)
    n_ = np.arange(32)
    for k in range(16):
        md[n_, 32 * k + n_] = 1.0 / 32.0
    return e16, m32r, md


def _build_nc(t_steps=T):
    nc = bacc.Bacc(trn_type="TRN2", target_bir_lowering=False, debug=False)

    ap_xT = nc.dram_tensor("xT", [D, T * NL], BF16, kind="ExternalInput").ap()
    ap_Asc = nc.dram_tensor("Asc", [H, 512], BF16, kind="ExternalInput").ap()
    ap_Asc32 = nc.dram_tensor("Asc32", [H, 512], F32, kind="ExternalInput").ap()
    ap_Wx = nc.dram_tensor("Wx", [D, H4], BF16, kind="ExternalInput").ap()
    ap_Wh = nc.dram_tensor("Wh", [H, H4], BF16, kind="ExternalInput").ap()
    ap_Wattn = nc.dram_tensor("Wattn", [H, H4], BF16, kind="ExternalInput").ap()
    ap_bT = nc.dram_tensor("bT", [128, 32], F32, kind="ExternalInput").ap()
    outT = nc.dram_tensor("outT", [T, H, NL], F32, kind="ExternalOutput").ap()
    # xps[t, r, p, q, j, n] = xproj[t][n, j*1024 + r*512 + q*128 + p]
    xps = nc.dram_tensor("xps", [T, 2, 128, 4, 4, NL], BF16, kind="Internal").ap()

    e16_np, m32r_np, md_np = _consts()
    eye_d = nc.inline_tensor(np.eye(128, dtype=BF), "c_eye")
    e16_d = nc.inline_tensor(e16_np, "c_e16")
    m32r_d = nc.inline_tensor(m32r_np, "c_m32r")
    md_d = nc.inline_tensor(md_np, "c_mdiag")

    with TileContext(nc) as tc:
        with tc.tile_pool(name="pers", bufs=1) as pers:
            Wh_sb = pers.tile([128, HC * H4], BF16, tag="Wh")
            Asc_sb = pers.tile([128, HC * 512], BF16, tag="Asc")
            P_sb = pers.tile([128, G * H4], BF16, tag="P")
            uTh = pers.tile([128, HC * 32], BF16, tag="uTh")
            cT = pers.tile([128, 256], F32, tag="cT")
            eye = pers.tile([128, 128], BF16, tag="eye")
            E16 = pers.tile([16, 128], BF16, tag="E16")
            M32R = pers.tile([128, 512], BF16, tag="M32R")
            Mdiag = pers.tile([32, 512], F32, tag="Mdiag")
            b_sb = pers.tile([128, 32], F32, tag="bT")
            wsq = pers.tile([32, 32], F32, tag="wsq")

            nc.sync.dma_start(eye[:], eye_d.ap()[:])
            nc.sync.dma_start(E16[:], e16_d.ap()[:])
            nc.sync.dma_start(M32R[:], m32r_d.ap()[:])
            nc.sync.dma_start(Mdiag[:], md_d.ap()[:])
            nc.sync.dma_start(b_sb[:], ap_bT[:])
            nc.gpsimd.memset(wsq[:], 0.0)
            for c in range(HC):
                nc.sync.dma_start(
                    Wh_sb[:, c * H4 : (c + 1) * H4], ap_Wh[128 * c : 128 * (c + 1), :]
                )
                nc.sync.dma_start(
                    Asc_sb[:, c * 512 : (c + 1) * 512],
                    ap_Asc[128 * c : 128 * (c + 1), :],
                )

            # ---------------- phase A: xproj -> DRAM scratch ----------------
            with tc.tile_pool(name="phx1", bufs=1) as phx1, \
                 tc.tile_pool(name="phx", bufs=3) as phx, \
                 tc.tile_pool(name="psX", bufs=2, space="PSUM") as psX:
                xT_sb = phx1.tile([128, HC * T * NL], BF16, tag="xTsb")
                for c in range(HC):
                    nc.sync.dma_start(
                        xT_sb[:, c * T * NL : (c + 1) * T * NL],
                        ap_xT[128 * c : 128 * (c + 1), :],
                    )
                for W in range(32):
                    j, r, q = W // 8, (W % 8) // 4, W % 4
                    Wxb = phx.tile([128, HC * 128], BF16, tag="Wxb")
                    for c in range(HC):
                        nc.sync.dma_start(
                            Wxb[:, c * 128 : (c + 1) * 128],
                            ap_Wx[128 * c : 128 * (c + 1), 128 * W : 128 * (W + 1)],
                        )
                    for t4 in range(4):
                        psx = psX.tile([128, 512], F32, tag="psx")
                        for c in range(HC):
                            nc.tensor.matmul(
                                psx[:],
                                Wxb[:, c * 128 : (c + 1) * 128],
                                xT_sb[:, c * T * NL + 512 * t4 : c * T * NL + 512 * (t4 + 1)],
                                start=(c == 0),
                                stop=(c == HC - 1),
                            )
                        sxp = phx.tile([128, 512], BF16, tag="sxp")
                        nc.vector.tensor_scalar_add(sxp[:], psx[:], b_sb[:, W : W + 1])
                        nc.sync.dma_start(
                            xps[16 * t4 : 16 * (t4 + 1), r, :, q, j, :].transpose(
                                [1, 0, 2]
                            ),
                            sxp[:].rearrange("p (t n) -> p t n", t=16),
                        )

            # ------------- phase B: P precompute + h0/c0 init -------------
            with tc.tile_pool(name="php1", bufs=1) as php1, \
                 tc.tile_pool(name="php", bufs=3) as php, \
                 tc.tile_pool(name="psP", bufs=2, space="PSUM") as psP:
                A32 = php1.tile([128, HC * 512], F32, tag="A32")
                for c in range(HC):
                    nc.sync.dma_start(
                        A32[:, c * 512 : (c + 1) * 512],
                        ap_Asc32[128 * c : 128 * (c + 1), :],
                    )
                for c in range(HC):
                    h0s = php.tile([128, 32], F32, tag="h0s")
                    nc.vector.tensor_reduce(
                        h0s[:],
                        A32[:, c * 512 : (c + 1) * 512].rearrange(
                            "p (k n) -> p n k", k=16
                        ),
                        axis=AX.X,
                        op=ALU.add,
                    )
                    nc.vector.tensor_scalar_mul(
                        cT[:, 32 * c : 32 * (c + 1)], h0s[:], 1.0 / 16.0
                    )
                    nc.vector.tensor_copy(
                        uTh[:, 32 * c : 32 * (c + 1)],
                        cT[:, 32 * c : 32 * (c + 1)],
                    )
                # contiguous staging of the group-selected A columns so the
                # matmul stationary operand has a single free dim
                Ag = php1.tile([128, G * HC * 128], BF16, tag="Ag")
                for g in range(G):
                    for c in range(HC):
                        nc.vector.tensor_copy(
                            Ag[:, (g * HC + c) * 128 : (g * HC + c) * 128 + 128],
                            Asc_sb[:, c * 512 : (c + 1) * 512].rearrange(
                                "p (k n) -> p k n", k=16
                            )[:, :, GS * g : GS * (g + 1)],
                        )
                for blk in range(8):
                    Wab = php.tile([128, HC * 512], BF16, tag="Wab")
                    for c in range(HC):
                        nc.sync.dma_start(
                            Wab[:, c * 512 : (c + 1) * 512],
                            ap_Wattn[128 * c : 128 * (c + 1), 512 * blk : 512 * (blk + 1)],
                        )
                    for g in range(G):
                        psp = psP.tile([128, 512], F32, tag="psp")
                        for c in range(HC):
                            nc.tensor.matmul(
                                psp[:],
                                Ag[:, (g * HC + c) * 128 : (g * HC + c) * 128 + 128],
                                Wab[:, c * 512 : (c + 1) * 512],
                                start=(c == 0),
                                stop=(c == HC - 1),
                            )
                        nc.vector.tensor_copy(
                            P_sb[:, g * H4 + 512 * blk : g * H4 + 512 * (blk + 1)],
                            psp[:],
                        )

            # ---------------------- phase C: recurrence ----------------------
            with tc.tile_pool(name="wrk", bufs=2) as wrk, \
                 tc.tile_pool(name="psc", bufs=2, space="PSUM") as psc_pool, \
                 tc.tile_pool(name="pwx", bufs=1, space="PSUM") as pwx_pool, \
                 tc.tile_pool(name="pstr", bufs=1, space="PSUM") as pstr_pool, \
                 tc.tile_pool(name="paT", bufs=1, space="PSUM") as paT_pool:
                q4 = lambda ap: ap.rearrange("p (q c) -> p q c", q=4)
                for t in range(t_steps):
                    # prefetched xproj slices for this step
                    xpt = [wrk.tile([128, 512], BF16, tag=f"xpt{r}", name=f"xpt{r}_{t}") for r in range(2)]
                    for r in range(2):
                        nc.sync.dma_start(xpt[r][:], xps[t, r])

                    # -- scores: cross-sample products, diag extract, softmax
                    psc = psc_pool.tile([32, 512], F32, tag="psc")
                    for c in range(HC):
                        nc.tensor.matmul(
                            psc[:],
                            uTh[:, c * 32 : (c + 1) * 32],
                            Asc_sb[:, c * 512 : (c + 1) * 512],
                            start=(c == 0),
                            stop=(c == HC - 1),
                        )
                    scm = wrk.tile([32, 512], F32, tag="scm")
                    nc.vector.tensor_mul(scm[:], psc[:], Mdiag[:])
                    scores = wrk.tile([32, 16], F32, tag="scores")
                    nc.vector.tensor_reduce(
                        scores[:],
                        scm[:].rearrange("p (k n) -> p k n", k=16),
                        axis=AX.X,
                        op=ALU.add,
                    )
                    nmx = wrk.tile([32, 1], F32, tag="nmx")
                    nc.vector.tensor_reduce(
                        nmx[:], scores[:], axis=AX.X, op=ALU.max, negate=True
                    )
                    ex = wrk.tile([32, 16], F32, tag="ex")
                    esum = wrk.tile([32, 1], F32, tag="esum")
                    nc.scalar.activation(
                        ex[:], scores[:], AF.Exp, bias=nmx[:], scale=1.0,
                        accum_out=esum[:],
                    )
                    rcp = wrk.tile([32, 1], F32, tag="rcp")
                    nc.vector.reciprocal(rcp[:], esum[:])
                    nc.vector.tensor_scalar_mul(wsq[:, 0:16], ex[:], rcp[:])
                    wT = wrk.tile([32, 32], F32, tag="wT")
                    nc.vector.transpose(wT[:], wsq[:])
                    wrep = wrk.tile([16, 128], BF16, tag="wrep")
                    for rep in range(4):
                        nc.vector.tensor_copy(
                            wrep[:, 32 * rep : 32 * (rep + 1)], wT[0:16, 0:32]
                        )
                    pwx = pwx_pool.tile([128, 128], F32, tag="pwx")
                    nc.tensor.matmul(pwx[:], E16[:], wrep[:], start=True, stop=True)
                    masked = wrk.tile([128, 512], BF16, tag="masked")
                    for g in range(G):
                        nc.vector.tensor_mul(
                            masked[:, g * 128 : (g + 1) * 128],
                            pwx[:],
                            M32R[:, g * 128 : (g + 1) * 128],
                        )

                    # -- gates: h @ Wh + sum_k w_k P_k into 2 column-tiled strips
                    strips = [
                        pstr_pool.tile([128, 512], F32, tag=f"strip{r}",
                                       name=f"strip{r}_{t}")
                        for r in range(2)
                    ]
                    for c in range(HC):
                        for r in range(2):
                            for j in range(4):
                                nc.tensor.matmul(
                                    strips[r][32 * j : 32 * (j + 1), :],
                                    uTh[:, c * 32 : (c + 1) * 32],
                                    Wh_sb[:, c * H4 + j * 1024 + r * 512 : c * H4 + j * 1024 + r * 512 + 512],
                                    start=(c == 0),
                                    stop=False,
                                    skip_group_check=True,
                                    tile_position=(0, 32 * j),
                                )
                    for r in range(2):
                        for g in range(G):
                            for j in range(4):
                                nc.tensor.matmul(
                                    strips[r][32 * j : 32 * (j + 1), :],
                                    masked[:, g * 128 + 32 * j : g * 128 + 32 * (j + 1)],
                                    P_sb[:, g * H4 + j * 1024 + r * 512 : g * H4 + j * 1024 + r * 512 + 512],
                                    start=False,
                                    stop=(g == G - 1),
                                    skip_group_check=True,
                                    tile_position=(0, 32 * j),
                                )

                    # -- transpose strips, add xproj, activations, cell update
                    for r in range(2):
                        sg = wrk.tile([128, 512], BF16, tag=f"sg{r}")
                        nc.vector.tensor_copy(sg[:], strips[r][:])
                        pat = paT_pool.tile([128, 512], BF16, tag=f"pat{r}")
                        for q in range(4):
                            nc.tensor.matmul(
                                pat[:, 128 * q : 128 * (q + 1)],
                                sg[:, 128 * q : 128 * (q + 1)],
                                eye[:],
                                is_transpose=True,
                                start=(q == 0),
                                stop=(q == 3),
                            )
                        ssum = wrk.tile([128, 512], F32, tag=f"ssum{r}")
                        nc.vector.tensor_add(ssum[:], pat[:], xpt[r][:])
                        act = wrk.tile([128, 512], F32, tag=f"act{r}")
                        nc.scalar.activation(
                            q4(act[:])[:, :, 0:96], q4(ssum[:])[:, :, 0:96], AF.Sigmoid
                        )
                        nc.scalar.activation(
                            q4(act[:])[:, :, 96:128], q4(ssum[:])[:, :, 96:128], AF.Tanh
                        )
                        i_v = q4(act[:])[:, :, 0:32]
                        f_v = q4(act[:])[:, :, 32:64]
                        o_v = q4(act[:])[:, :, 64:96]
                        g_v = q4(act[:])[:, :, 96:128]
                        cview = cT[:, 128 * r : 128 * (r + 1)].rearrange(
                            "p (q n) -> p q n", q=4
                        )
                        ig = wrk.tile([128, 128], F32, tag=f"ig{r}")
                        nc.vector.tensor_mul(q4(ig[:]), i_v, g_v)
                        fc = wrk.tile([128, 128], F32, tag=f"fc{r}")
                        nc.vector.tensor_mul(q4(fc[:]), f_v, cview)
                        nc.vector.tensor_add(
                            cview, q4(ig[:]), q4(fc[:])
                        )
                        tch = wrk.tile([128, 128], F32, tag=f"tch{r}")
                        nc.scalar.activation(
                            tch[:], cT[:, 128 * r : 128 * (r + 1)], AF.Tanh
                        )
                        h32 = wrk.tile([128, 128], F32, tag=f"h32{r}")
                        nc.vector.tensor_mul(
                            h32[:].rearrange("p (q n) -> p q n", q=4),
                            o_v,
                            tch[:].rearrange("p (q n) -> p q n", q=4),
                        )
                        # write h into uTh (bf16) for step t+1
                        nc.vector.tensor_copy(
                            uTh[:, 128 * r : 128 * (r + 1)], h32[:]
                        )
                        nc.sync.dma_start(
                            outT[t, 512 * r : 512 * (r + 1), :].rearrange(
                                "(q p) n -> p q n", p=128
                            ),
                            h32[:].rearrange("p (q n) -> p q n", q=4),
                        )
    nc.compile()
    return nc


def _prep_shards(inputs):
    x = np.asarray(inputs["x"], np.float32)
    A = np.asarray(inputs["A"], np.float32)
    Wx = np.asarray(inputs["Wx"], np.float32)
    Wh = np.asarray(inputs["Wh"], np.float32)
    Wattn = np.asarray(inputs["Wattn"], np.float32)
    b = np.asarray(inputs["b"], np.float32)

    Wx_bf = np.ascontiguousarray(Wx.astype(BF))
    Wh_bf = np.ascontiguousarray(Wh.astype(BF))
    Wa_bf = np.ascontiguousarray(Wattn.astype(BF))
    bT = np.ascontiguousarray(b.reshape(32, 128).T.astype(np.float32))

    in_maps = []
    for i in range(NCORES):
        ns = slice(NL * i, NL * (i + 1))
        xT = x[ns].transpose(2, 1, 0).reshape(D, T * NL)
        Asc = A[ns].reshape(NL, H, 16).transpose(1, 2, 0).reshape(H, 512)
        in_maps.append(
            {
                "xT": np.ascontiguousarray(xT.astype(BF)),
                "Asc": np.ascontiguousarray(Asc.astype(BF)),
                "Asc32": np.ascontiguousarray(Asc.astype(np.float32)),
                "Wx": Wx_bf,
                "Wh": Wh_bf,
                "Wattn": Wa_bf,
                "bT": bT,
            }
        )
    return in_maps


def _get_nc():
    global _built
    if _built is None:
        _built = _build_nc()
    return _built


def _run(inputs, **kwargs):
    nc = _get_nc()
    in_maps = _prep_shards(inputs)
    res = bass_utils.run_bass_kernel_spmd(
        nc, in_maps, core_ids=list(range(NCORES)), **kwargs
    )
    out = np.empty((N, T, H), np.float32)
    for i in range(NCORES):
        out[NL * i : NL * (i + 1)] = res.results[i]["outT"].transpose(2, 0, 1)
    return out, res


def kernel(**inputs):
    out, _ = _run(inputs)
    return out



# revision 14
# speedup vs baseline: 1.1355x; 1.0864x over previous
"""Trainium2 Bass kernel for nn_CaptioningRNN (attention LSTM over T=64).

Data-parallel over the batch: N=256 samples split across 8 NeuronCores
(32 samples/core), weights replicated, no collectives.

Per-core algorithm (all matmuls bf16 on the TensorEngine, state in f32):
  1. xproj phase: xpT = (x @ Wx + b) computed transposed via Wx-stationary
     matmuls, stored to a DRAM scratch laid out so the per-step slice loads
     as a clean [128, 512] tile.
  2. P phase: P[n, k, :] = A[n, :, k] @ Wattn precomputed once (the
     attention context contribution to the gates becomes a w-weighted sum
     of P rows, replacing a per-step [32,1024]@[1024,4096] matmul).
     h0 = c0 = mean_k(A) computed on device from an f32 copy of A.
  3. Recurrence (64 steps):
     - scores via hT-chunk matmuls against a permuted A (cross-sample
       products in PSUM, diagonal extracted with a mask+reduce on DVE)
     - softmax on [32,16] (ACT exp with fused row-sum)
     - w transposed (DVE 32x32 stream transpose) and expanded to the
       (k, n_g)-partition block-diagonal layout via a one-hot matmul + mask
     - gates = h @ Wh + sum_k w_k P_k accumulated into two PSUM strips
       using 4-way tensor-engine column tiling (beats the M=32 small-batch
       penalty ~4x)
     - strips transposed on PE; cell math done in h-on-partition space so
       i/f/o/g land on identical lanes (no cross-partition ops needed)
  4. Output written transposed [t, h, n]; host reassembles to (N, T, H).
"""

import numpy as np
import ml_dtypes

import concourse.bacc as bacc
import concourse.mybir as mybir
from concourse import bass_utils
from concourse.tile import TileContext

F32, BF16 = mybir.dt.float32, mybir.dt.bfloat16
AF = mybir.ActivationFunctionType
ALU = mybir.AluOpType
AX = mybir.AxisListType
BF = ml_dtypes.bfloat16

N, T, D, H = 256, 64, 1024, 1024
NCORES = 8
NL = N // NCORES          # 32 samples per core
HC = 8                    # 128-row chunks of D/H
G, GS = 4, 8              # sample groups of 8 (for the (k, n_g) 128-partition layout)
H4 = 4 * H                # 4096 gate columns

_built = None


def _consts():
    # E16[k', 8k + n] = (k' == k): one-hot expansion of wT rows onto the
    # (k-major, n_g-minor) 128-partition layout.
    e16 = np.zeros((16, 128), dtype=BF)
    for k in range(16):
        e16[k, 8 * k : 8 * k + 8] = 1
    # M32R[p, 128 g + 32 rep + m] = (m % 8 == p % 8) & (m // 8 == g):
    # block-diagonal mask producing masked_g = w[m, k(p)] only for group-g
    # samples, replicated 4x for the column-tiled matmuls.
    p = np.arange(128)[:, None]
    m = np.arange(32)[None, :]
    m32r = np.zeros((128, 512), dtype=BF)
    for g in range(4):
        blk = ((m % 8 == p % 8) & (m // 8 == g)).astype(BF)
        for rep in range(4):
            m32r[:, 128 * g + 32 * rep : 128 * g + 32 * rep + 32] = blk
    # Mdiag[n, 32 k + n'] = (n == n') / 32: extracts the diagonal of the
    # cross-sample score products and applies the 1/sqrt(H) softmax scale.
    md = np.zeros((32, 512), dtype=np.float32)
    n_ = np.arange(32)
    for k in range(16):
        md[n_, 32 * k + n_] = 1.0 / 32.0
    return e16, m32r, md


def _build_nc(t_steps=T):
    nc = bacc.Bacc(trn_type="TRN2", target_bir_lowering=False, debug=False)

    ap_xT = nc.dram_tensor("xT", [D, T * NL], BF16, kind="ExternalInput").ap()
    ap_Asc = nc.dram_tensor("Asc", [H, 512], BF16, kind="ExternalInput").ap()
    ap_Asc32 = nc.dram_tensor("Asc32", [H, 512], F32, kind="ExternalInput").ap()
    ap_Wx = nc.dram_tensor("Wx", [D, H4], BF16, kind="ExternalInput").ap()
    ap_Wh = nc.dram_tensor("Wh", [H, H4], BF16, kind="ExternalInput").ap()
    ap_Wattn = nc.dram_tensor("Wattn", [H, H4], BF16, kind="ExternalInput").ap()
    ap_bT = nc.dram_tensor("bT", [128, 32], F32, kind="ExternalInput").ap()
    outT = nc.dram_tensor("outT", [T, H, NL], F32, kind="ExternalOutput").ap()
    # xps[t, r, p, q, j, n] = xproj[t][n, j*1024 + r*512 + q*128 + p]
    xps = nc.dram_tensor("xps", [T, 2, 128, 4, 4, NL], BF16, kind="Internal").ap()

    e16_np, m32r_np, md_np = _consts()
    eye_d = nc.inline_tensor(np.eye(128, dtype=BF), "c_eye")
    e16_d = nc.inline_tensor(e16_np, "c_e16")
    m32r_d = nc.inline_tensor(m32r_np, "c_m32r")
    md_d = nc.inline_tensor(md_np, "c_mdiag")

    with TileContext(nc) as tc:
        with tc.tile_pool(name="pers", bufs=1) as pers:
            Wh_sb = pers.tile([128, HC * H4], BF16, tag="Wh")
            Asc_sb = pers.tile([128, HC * 512], BF16, tag="Asc")
            P_sb = pers.tile([128, G * H4], BF16, tag="P")
            uTh = pers.tile([128, HC * 32], BF16, tag="uTh")
            cT = pers.tile([128, 256], F32, tag="cT")
            eye = pers.tile([128, 128], BF16, tag="eye")
            E16 = pers.tile([16, 128], BF16, tag="E16")
            M32R = pers.tile([128, 512], BF16, tag="M32R")
            Mdiag = pers.tile([32, 512], F32, tag="Mdiag")
            b_sb = pers.tile([128, 32], F32, tag="bT")
            wsq = pers.tile([32, 32], F32, tag="wsq")

            nc.sync.dma_start(eye[:], eye_d.ap()[:])
            nc.sync.dma_start(E16[:], e16_d.ap()[:])
            nc.sync.dma_start(M32R[:], m32r_d.ap()[:])
            nc.sync.dma_start(Mdiag[:], md_d.ap()[:])
            nc.sync.dma_start(b_sb[:], ap_bT[:])
            nc.gpsimd.memset(wsq[:], 0.0)
            for c in range(HC):
                nc.sync.dma_start(
                    Wh_sb[:, c * H4 : (c + 1) * H4], ap_Wh[128 * c : 128 * (c + 1), :]
                )
                nc.sync.dma_start(
                    Asc_sb[:, c * 512 : (c + 1) * 512],
                    ap_Asc[128 * c : 128 * (c + 1), :],
                )

            # ---------------- phase A: xproj -> DRAM scratch ----------------
            with tc.tile_pool(name="phx1", bufs=1) as phx1, \
                 tc.tile_pool(name="phx", bufs=3) as phx, \
                 tc.tile_pool(name="psX", bufs=2, space="PSUM") as psX:
                xT_sb = phx1.tile([128, HC * T * NL], BF16, tag="xTsb")
                for c in range(HC):
                    nc.sync.dma_start(
                        xT_sb[:, c * T * NL : (c + 1) * T * NL],
                        ap_xT[128 * c : 128 * (c + 1), :],
                    )
                for W in range(32):
                    j, r, q = W // 8, (W % 8) // 4, W % 4
                    Wxb = phx.tile([128, HC * 128], BF16, tag="Wxb")
                    for c in range(HC):
                        nc.sync.dma_start(
                            Wxb[:, c * 128 : (c + 1) * 128],
                            ap_Wx[128 * c : 128 * (c + 1), 128 * W : 128 * (W + 1)],
                        )
                    for t4 in range(4):
                        psx = psX.tile([128, 512], F32, tag="psx")
                        for c in range(HC):
                            nc.tensor.matmul(
                                psx[:],
                                Wxb[:, c * 128 : (c + 1) * 128],
                                xT_sb[:, c * T * NL + 512 * t4 : c * T * NL + 512 * (t4 + 1)],
                                start=(c == 0),
                                stop=(c == HC - 1),
                            )
                        sxp = phx.tile([128, 512], BF16, tag="sxp")
                        nc.vector.tensor_scalar_add(sxp[:], psx[:], b_sb[:, W : W + 1])
                        nc.sync.dma_start(
                            xps[16 * t4 : 16 * (t4 + 1), r, :, q, j, :].transpose(
                                [1, 0, 2]
                            ),
                            sxp[:].rearrange("p (t n) -> p t n", t=16),
                        )

            # ------------- phase B: P precompute + h0/c0 init -------------
            with tc.tile_pool(name="php1", bufs=1) as php1, \
                 tc.tile_pool(name="php", bufs=3) as php, \
                 tc.tile_pool(name="psP", bufs=2, space="PSUM") as psP:
                A32 = php1.tile([128, HC * 512], F32, tag="A32")
                for c in range(HC):
                    nc.sync.dma_start(
                        A32[:, c * 512 : (c + 1) * 512],
                        ap_Asc32[128 * c : 128 * (c + 1), :],
                    )
                for c in range(HC):
                    h0s = php.tile([128, 32], F32, tag="h0s")
                    nc.vector.tensor_reduce(
                        h0s[:],
                        A32[:, c * 512 : (c + 1) * 512].rearrange(
                            "p (k n) -> p n k", k=16
                        ),
                        axis=AX.X,
                        op=ALU.add,
                    )
                    nc.vector.tensor_scalar_mul(
                        cT[:, 32 * c : 32 * (c + 1)], h0s[:], 1.0 / 16.0
                    )
                    nc.vector.tensor_copy(
                        uTh[:, 32 * c : 32 * (c + 1)],
                        cT[:, 32 * c : 32 * (c + 1)],
                    )
                # contiguous staging of the group-selected A columns so the
                # matmul stationary operand has a single free dim
                Ag = php1.tile([128, G * HC * 128], BF16, tag="Ag")
                for g in range(G):
                    for c in range(HC):
                        nc.vector.tensor_copy(
                            Ag[:, (g * HC + c) * 128 : (g * HC + c) * 128 + 128],
                            Asc_sb[:, c * 512 : (c + 1) * 512].rearrange(
                                "p (k n) -> p k n", k=16
                            )[:, :, GS * g : GS * (g + 1)],
                        )
                for blk in range(8):
                    Wab = php.tile([128, HC * 512], BF16, tag="Wab")
                    for c in range(HC):
                        nc.sync.dma_start(
                            Wab[:, c * 512 : (c + 1) * 512],
                            ap_Wattn[128 * c : 128 * (c + 1), 512 * blk : 512 * (blk + 1)],
                        )
                    for g in range(G):
                        psp = psP.tile([128, 512], F32, tag="psp")
                        for c in range(HC):
                            nc.tensor.matmul(
                                psp[:],
                                Ag[:, (g * HC + c) * 128 : (g * HC + c) * 128 + 128],
                                Wab[:, c * 512 : (c + 1) * 512],
                                start=(c == 0),
                                stop=(c == HC - 1),
                            )
                        nc.vector.tensor_copy(
                            P_sb[:, g * H4 + 512 * blk : g * H4 + 512 * (blk + 1)],
                            psp[:],
                        )

            # ---------------------- phase C: recurrence ----------------------
            with tc.tile_pool(name="wrk", bufs=2) as wrk, \
                 tc.tile_pool(name="psc", bufs=2, space="PSUM") as psc_pool, \
                 tc.tile_pool(name="pwx", bufs=1, space="PSUM") as pwx_pool, \
                 tc.tile_pool(name="pstr", bufs=1, space="PSUM") as pstr_pool, \
                 tc.tile_pool(name="paT", bufs=1, space="PSUM") as paT_pool:
                q4 = lambda ap: ap.rearrange("p (q c) -> p q c", q=4)
                for t in range(t_steps):
                    # prefetched xproj slices for this step
                    xpt = [wrk.tile([128, 512], BF16, tag=f"xpt{r}", name=f"xpt{r}_{t}") for r in range(2)]
                    for r in range(2):
                        nc.sync.dma_start(xpt[r][:], xps[t, r])

                    # -- scores: cross-sample products, diag extract, softmax
                    psc = psc_pool.tile([32, 512], F32, tag="psc")
                    for c in range(HC):
                        nc.tensor.matmul(
                            psc[:],
                            uTh[:, c * 32 : (c + 1) * 32],
                            Asc_sb[:, c * 512 : (c + 1) * 512],
                            start=(c == 0),
                            stop=(c == HC - 1),
                        )
                    scm = wrk.tile([32, 512], F32, tag="scm")
                    nc.vector.tensor_mul(scm[:], psc[:], Mdiag[:])
                    scores = wrk.tile([32, 16], F32, tag="scores")
                    nc.vector.tensor_reduce(
                        scores[:],
                        scm[:].rearrange("p (k n) -> p k n", k=16),
                        axis=AX.X,
                        op=ALU.add,
                    )
                    nmx = wrk.tile([32, 1], F32, tag="nmx")
                    nc.vector.tensor_reduce(
                        nmx[:], scores[:], axis=AX.X, op=ALU.max, negate=True
                    )
                    # softmax via the sigmoid table (keeps every ACT op in the
                    # sigmoid_and_others set -> one table load for the kernel):
                    # y = sigmoid(s - m) in (0, 0.5], e^(s-m) = y / (1 - y)
                    ysig = wrk.tile([32, 16], F32, tag="ysig")
                    nc.scalar.activation(
                        ysig[:], scores[:], AF.Sigmoid, bias=nmx[:], scale=1.0
                    )
                    omy = wrk.tile([32, 16], F32, tag="omy")
                    nc.vector.tensor_scalar(
                        omy[:], ysig[:], -1.0, 1.0, ALU.mult, ALU.add
                    )
                    romy = wrk.tile([32, 16], F32, tag="romy")
                    nc.vector.reciprocal(romy[:], omy[:])
                    ex = wrk.tile([32, 16], F32, tag="ex")
                    esum = wrk.tile([32, 1], F32, tag="esum")
                    nc.vector.scalar_tensor_tensor(
                        ex[:], ysig[:], 1.0, romy[:], ALU.mult, ALU.mult,
                        accum_out=esum[:],
                    )
                    rcp = wrk.tile([32, 1], F32, tag="rcp")
                    nc.vector.reciprocal(rcp[:], esum[:])
                    nc.vector.tensor_scalar_mul(wsq[:, 0:16], ex[:], rcp[:])
                    wT = wrk.tile([32, 32], F32, tag="wT")
                    nc.vector.transpose(wT[:], wsq[:])
                    wrep = wrk.tile([16, 128], BF16, tag="wrep")
                    for rep in range(4):
                        nc.vector.tensor_copy(
                            wrep[:, 32 * rep : 32 * (rep + 1)], wT[0:16, 0:32]
                        )
                    pwx = pwx_pool.tile([128, 128], F32, tag="pwx")
                    nc.tensor.matmul(pwx[:], E16[:], wrep[:], start=True, stop=True)
                    masked = wrk.tile([128, 512], BF16, tag="masked")
                    for g in range(G):
                        nc.vector.tensor_mul(
                            masked[:, g * 128 : (g + 1) * 128],
                            pwx[:],
                            M32R[:, g * 128 : (g + 1) * 128],
                        )

                    # -- gates: h @ Wh + sum_k w_k P_k into 2 column-tiled strips
                    strips = [
                        pstr_pool.tile([128, 512], F32, tag=f"strip{r}",
                                       name=f"strip{r}_{t}")
                        for r in range(2)
                    ]
                    for c in range(HC):
                        for r in range(2):
                            for j in range(4):
                                nc.tensor.matmul(
                                    strips[r][32 * j : 32 * (j + 1), :],
                                    uTh[:, c * 32 : (c + 1) * 32],
                                    Wh_sb[:, c * H4 + j * 1024 + r * 512 : c * H4 + j * 1024 + r * 512 + 512],
                                    start=(c == 0),
                                    stop=False,
                                    skip_group_check=True,
                                    tile_position=(0, 32 * j),
                                )
                    for r in range(2):
                        for g in range(G):
                            for j in range(4):
                                nc.tensor.matmul(
                                    strips[r][32 * j : 32 * (j + 1), :],
                                    masked[:, g * 128 + 32 * j : g * 128 + 32 * (j + 1)],
                                    P_sb[:, g * H4 + j * 1024 + r * 512 : g * H4 + j * 1024 + r * 512 + 512],
                                    start=False,
                                    stop=(g == G - 1),
                                    skip_group_check=True,
                                    tile_position=(0, 32 * j),
                                )

                    # -- transpose strips, add xproj, activations, cell update
                    for r in range(2):
                        sg = wrk.tile([128, 512], BF16, tag=f"sg{r}")
                        nc.vector.tensor_copy(sg[:], strips[r][:])
                        pat = paT_pool.tile([128, 512], BF16, tag=f"pat{r}")
                        for q in range(4):
                            nc.tensor.matmul(
                                pat[:, 128 * q : 128 * (q + 1)],
                                sg[:, 128 * q : 128 * (q + 1)],
                                eye[:],
                                is_transpose=True,
                                start=(q == 0),
                                stop=(q == 3),
                            )
                        ssum = wrk.tile([128, 512], F32, tag=f"ssum{r}")
                        nc.vector.tensor_add(ssum[:], pat[:], xpt[r][:])
                        act = wrk.tile([128, 512], F32, tag=f"act{r}")
                        nc.scalar.activation(
                            q4(act[:])[:, :, 0:96], q4(ssum[:])[:, :, 0:96], AF.Sigmoid
                        )
                        nc.scalar.activation(
                            q4(act[:])[:, :, 96:128], q4(ssum[:])[:, :, 96:128], AF.Tanh
                        )
                        i_v = q4(act[:])[:, :, 0:32]
                        f_v = q4(act[:])[:, :, 32:64]
                        o_v = q4(act[:])[:, :, 64:96]
                        g_v = q4(act[:])[:, :, 96:128]
                        cview = cT[:, 128 * r : 128 * (r + 1)].rearrange(
                            "p (q n) -> p q n", q=4
                        )
                        ig = wrk.tile([128, 128], F32, tag=f"ig{r}")
                        nc.vector.tensor_mul(q4(ig[:]), i_v, g_v)
                        fc = wrk.tile([128, 128], F32, tag=f"fc{r}")
                        nc.vector.tensor_mul(q4(fc[:]), f_v, cview)
                        nc.vector.tensor_add(
                            cview, q4(ig[:]), q4(fc[:])
                        )
                        tch = wrk.tile([128, 128], F32, tag=f"tch{r}")
                        nc.scalar.activation(
                            tch[:], cT[:, 128 * r : 128 * (r + 1)], AF.Tanh
                        )
                        h32 = wrk.tile([128, 128], F32, tag=f"h32{r}")
                        nc.vector.tensor_mul(
                            h32[:].rearrange("p (q n) -> p q n", q=4),
                            o_v,
                            tch[:].rearrange("p (q n) -> p q n", q=4),
                        )
                        # write h into uTh (bf16) for step t+1
                        nc.vector.tensor_copy(
                            uTh[:, 128 * r : 128 * (r + 1)], h32[:]
                        )
                        nc.sync.dma_start(
                            outT[t, 512 * r : 512 * (r + 1), :].rearrange(
                                "(q p) n -> p q n", p=128
                            ),
                            h32[:].rearrange("p (q n) -> p q n", q=4),
                        )
    nc.compile()
    return nc


def _prep_shards(inputs):
    x = np.asarray(inputs["x"], np.float32)
    A = np.asarray(inputs["A"], np.float32)
    Wx = np.asarray(inputs["Wx"], np.float32)
    Wh = np.asarray(inputs["Wh"], np.float32)
    Wattn = np.asarray(inputs["Wattn"], np.float32)
    b = np.asarray(inputs["b"], np.float32)

    Wx_bf = np.ascontiguousarray(Wx.astype(BF))
    Wh_bf = np.ascontiguousarray(Wh.astype(BF))
    Wa_bf = np.ascontiguousarray(Wattn.astype(BF))
    bT = np.ascontiguousarray(b.reshape(32, 128).T.astype(np.float32))

    in_maps = []
    for i in range(NCORES):
        ns = slice(NL * i, NL * (i + 1))
        xT = x[ns].transpose(2, 1, 0).reshape(D, T * NL)
        Asc = A[ns].reshape(NL, H, 16).transpose(1, 2, 0).reshape(H, 512)
        in_maps.append(
            {
                "xT": np.ascontiguousarray(xT.astype(BF)),
                "Asc": np.ascontiguousarray(Asc.astype(BF)),
                "Asc32": np.ascontiguousarray(Asc.astype(np.float32)),
                "Wx": Wx_bf,
                "Wh": Wh_bf,
                "Wattn": Wa_bf,
                "bT": bT,
            }
        )
    return in_maps


def _get_nc():
    global _built
    if _built is None:
        _built = _build_nc()
    return _built


def _run(inputs, **kwargs):
    nc = _get_nc()
    in_maps = _prep_shards(inputs)
    res = bass_utils.run_bass_kernel_spmd(
        nc, in_maps, core_ids=list(range(NCORES)), **kwargs
    )
    out = np.empty((N, T, H), np.float32)
    for i in range(NCORES):
        out[NL * i : NL * (i + 1)] = res.results[i]["outT"].transpose(2, 0, 1)
    return out, res


def kernel(**inputs):
    out, _ = _run(inputs)
    return out



# revision 23
# speedup vs baseline: 1.1793x; 1.0385x over previous
"""Trainium2 Bass kernel for nn_CaptioningRNN (attention LSTM over T=64).

Data-parallel over the batch: N=256 samples split across 8 NeuronCores
(32 samples/core), weights replicated, no collectives.

Per-core algorithm (all matmuls bf16 on the TensorEngine, state in f32):
  1. xproj phase: xpT = (x @ Wx + b) computed transposed via Wx-stationary
     matmuls, stored to a DRAM scratch laid out so the per-step slice loads
     as a clean [128, 512] tile.
  2. P phase: P[n, k, :] = A[n, :, k] @ Wattn precomputed once (the
     attention context contribution to the gates becomes a w-weighted sum
     of P rows, replacing a per-step [32,1024]@[1024,4096] matmul).
     h0 = c0 = mean_k(A) computed on device from an f32 copy of A.
  3. Recurrence (64 steps):
     - scores via hT-chunk matmuls against a permuted A (cross-sample
       products in PSUM, diagonal extracted with a mask+reduce on DVE)
     - softmax on [32,16] (ACT exp with fused row-sum)
     - w transposed (DVE 32x32 stream transpose) and expanded to the
       (k, n_g)-partition block-diagonal layout via a one-hot matmul + mask
     - gates = h @ Wh + sum_k w_k P_k accumulated into two PSUM strips
       using 4-way tensor-engine column tiling (beats the M=32 small-batch
       penalty ~4x)
     - strips transposed on PE; cell math done in h-on-partition space so
       i/f/o/g land on identical lanes (no cross-partition ops needed)
  4. Output written transposed [t, h, n]; host reassembles to (N, T, H).
"""

import numpy as np
import ml_dtypes

import concourse.bacc as bacc
import concourse.mybir as mybir
from concourse import bass_utils
from concourse.tile import TileContext

F32, BF16 = mybir.dt.float32, mybir.dt.bfloat16
AF = mybir.ActivationFunctionType
ALU = mybir.AluOpType
AX = mybir.AxisListType
BF = ml_dtypes.bfloat16

N, T, D, H = 256, 64, 1024, 1024
NCORES = 8
NL = N // NCORES          # 32 samples per core
HC = 8                    # 128-row chunks of D/H
G, GS = 4, 8              # sample groups of 8 (for the (k, n_g) 128-partition layout)
H4 = 4 * H                # 4096 gate columns

_built = None


def _consts():
    # E16[k', 8k + n] = (k' == k): one-hot expansion of wT rows onto the
    # (k-major, n_g-minor) 128-partition layout.
    e16 = np.zeros((16, 128), dtype=BF)
    for k in range(16):
        e16[k, 8 * k : 8 * k + 8] = 1
    # M32R[p, 128 g + 32 rep + m] = (m % 8 == p % 8) & (m // 8 == g):
    # block-diagonal mask producing masked_g = w[m, k(p)] only for group-g
    # samples, replicated 4x for the column-tiled matmuls.
    p = np.arange(128)[:, None]
    m = np.arange(32)[None, :]
    m32r = np.zeros((128, 512), dtype=BF)
    for g in range(4):
        blk = ((m % 8 == p % 8) & (m // 8 == g)).astype(BF)
        for rep in range(4):
            m32r[:, 128 * g + 32 * rep : 128 * g + 32 * rep + 32] = blk
    # Mdiag8[32 g + m, 8 k + n] = (m == 8 g + n) / 32: extracts the
    # group-local diagonal of the score products (stationary = all 32
    # samples, moving = group-g A columns) and applies the 1/sqrt(H) scale.
    md8 = np.zeros((128, 128), dtype=np.float32)
    for g in range(4):
        for n in range(8):
            for k in range(16):
                md8[32 * g + 8 * g + n, 8 * k + n] = 1.0 / 32.0
    # selT[32 g + (8 g + n), 8 g + n] = 1: compacts the block-diagonal w
    # layout to wT[k, n] via a single PE matmul (stationary = w2).
    sel = np.zeros((128, 32), dtype=BF)
    for g in range(4):
        for n in range(8):
            sel[32 * g + 8 * g + n, 8 * g + n] = 1
    return e16, m32r, md8, sel


def _build_nc(t_steps=T):
    nc = bacc.Bacc(trn_type="TRN2", target_bir_lowering=False, debug=False)

    ap_xT = nc.dram_tensor("xT", [D, T * NL], BF16, kind="ExternalInput").ap()
    ap_Asc = nc.dram_tensor("Asc", [H, 512], BF16, kind="ExternalInput").ap()
    ap_Asc32 = nc.dram_tensor("Asc32", [H, 512], F32, kind="ExternalInput").ap()
    ap_Wx = nc.dram_tensor("Wx", [D, H4], BF16, kind="ExternalInput").ap()
    ap_Wh = nc.dram_tensor("Wh", [H, H4], BF16, kind="ExternalInput").ap()
    ap_Wattn = nc.dram_tensor("Wattn", [H, H4], BF16, kind="ExternalInput").ap()
    ap_bT = nc.dram_tensor("bT", [128, 32], F32, kind="ExternalInput").ap()
    outT = nc.dram_tensor("outT", [T, H, NL], F32, kind="ExternalOutput").ap()
    # xps[t, r, p, q, j, n] = xproj[t][n, j*1024 + r*512 + q*128 + p]
    xps = nc.dram_tensor("xps", [T, 2, 128, 4, 4, NL], BF16, kind="Internal").ap()

    e16_np, m32r_np, md8_np, sel_np = _consts()
    eye_d = nc.inline_tensor(np.eye(128, dtype=BF), "c_eye")
    e16_d = nc.inline_tensor(e16_np, "c_e16")
    m32r_d = nc.inline_tensor(m32r_np, "c_m32r")
    md8_d = nc.inline_tensor(md8_np, "c_mdiag8")
    sel_d = nc.inline_tensor(sel_np, "c_selT")

    with TileContext(nc) as tc:
        with tc.tile_pool(name="pers", bufs=1) as pers:
            Wh_sb = pers.tile([128, HC * H4], BF16, tag="Wh")
            Asc_sb = pers.tile([128, HC * 512], BF16, tag="Asc")
            P_sb = pers.tile([128, G * H4], BF16, tag="P")
            uTh = pers.tile([128, HC * 32], BF16, tag="uTh")
            cT = pers.tile([128, 256], F32, tag="cT")
            eye = pers.tile([128, 128], BF16, tag="eye")
            E16 = pers.tile([16, 128], BF16, tag="E16")
            M32R = pers.tile([128, 512], BF16, tag="M32R")
            Mdiag8 = pers.tile([128, 128], F32, tag="Mdiag8")
            selT = pers.tile([128, 32], BF16, tag="selT")
            b_sb = pers.tile([128, 32], F32, tag="bT")
            Ag = pers.tile([128, G * HC * 128], BF16, tag="Ag")

            nc.sync.dma_start(eye[:], eye_d.ap()[:])
            nc.sync.dma_start(E16[:], e16_d.ap()[:])
            nc.sync.dma_start(M32R[:], m32r_d.ap()[:])
            nc.sync.dma_start(Mdiag8[:], md8_d.ap()[:])
            nc.sync.dma_start(selT[:], sel_d.ap()[:])
            nc.sync.dma_start(b_sb[:], ap_bT[:])
            for c in range(HC):
                nc.sync.dma_start(
                    Wh_sb[:, c * H4 : (c + 1) * H4], ap_Wh[128 * c : 128 * (c + 1), :]
                )
                nc.sync.dma_start(
                    Asc_sb[:, c * 512 : (c + 1) * 512],
                    ap_Asc[128 * c : 128 * (c + 1), :],
                )

            # ---------------- phase A: xproj -> DRAM scratch ----------------
            with tc.tile_pool(name="phx1", bufs=1) as phx1, \
                 tc.tile_pool(name="phx", bufs=3) as phx, \
                 tc.tile_pool(name="psX", bufs=2, space="PSUM") as psX:
                xT_sb = phx1.tile([128, HC * T * NL], BF16, tag="xTsb")
                for c in range(HC):
                    nc.sync.dma_start(
                        xT_sb[:, c * T * NL : (c + 1) * T * NL],
                        ap_xT[128 * c : 128 * (c + 1), :],
                    )
                for W in range(32):
                    j, r, q = W // 8, (W % 8) // 4, W % 4
                    Wxb = phx.tile([128, HC * 128], BF16, tag="Wxb")
                    for c in range(HC):
                        nc.sync.dma_start(
                            Wxb[:, c * 128 : (c + 1) * 128],
                            ap_Wx[128 * c : 128 * (c + 1), 128 * W : 128 * (W + 1)],
                        )
                    for t4 in range(4):
                        psx = psX.tile([128, 512], F32, tag="psx")
                        for c in range(HC):
                            nc.tensor.matmul(
                                psx[:],
                                Wxb[:, c * 128 : (c + 1) * 128],
                                xT_sb[:, c * T * NL + 512 * t4 : c * T * NL + 512 * (t4 + 1)],
                                start=(c == 0),
                                stop=(c == HC - 1),
                            )
                        sxp = phx.tile([128, 512], BF16, tag="sxp")
                        nc.vector.tensor_scalar_add(sxp[:], psx[:], b_sb[:, W : W + 1])
                        nc.sync.dma_start(
                            xps[16 * t4 : 16 * (t4 + 1), r, :, q, j, :].transpose(
                                [1, 0, 2]
                            ),
                            sxp[:].rearrange("p (t n) -> p t n", t=16),
                        )

            # ------------- phase B: P precompute + h0/c0 init -------------
            with tc.tile_pool(name="php1", bufs=1) as php1, \
                 tc.tile_pool(name="php", bufs=3) as php, \
                 tc.tile_pool(name="psP", bufs=2, space="PSUM") as psP:
                A32 = php1.tile([128, HC * 512], F32, tag="A32")
                for c in range(HC):
                    nc.sync.dma_start(
                        A32[:, c * 512 : (c + 1) * 512],
                        ap_Asc32[128 * c : 128 * (c + 1), :],
                    )
                for c in range(HC):
                    h0s = php.tile([128, 32], F32, tag="h0s")
                    nc.vector.tensor_reduce(
                        h0s[:],
                        A32[:, c * 512 : (c + 1) * 512].rearrange(
                            "p (k n) -> p n k", k=16
                        ),
                        axis=AX.X,
                        op=ALU.add,
                    )
                    nc.vector.tensor_scalar_mul(
                        cT[:, 32 * c : 32 * (c + 1)], h0s[:], 1.0 / 16.0
                    )
                    nc.vector.tensor_copy(
                        uTh[:, 32 * c : 32 * (c + 1)],
                        cT[:, 32 * c : 32 * (c + 1)],
                    )
                # contiguous staging of the group-selected A columns so the
                # matmul stationary operand has a single free dim
                for g in range(G):
                    for c in range(HC):
                        nc.vector.tensor_copy(
                            Ag[:, (g * HC + c) * 128 : (g * HC + c) * 128 + 128],
                            Asc_sb[:, c * 512 : (c + 1) * 512].rearrange(
                                "p (k n) -> p k n", k=16
                            )[:, :, GS * g : GS * (g + 1)],
                        )
                for blk in range(8):
                    Wab = php.tile([128, HC * 512], BF16, tag="Wab")
                    for c in range(HC):
                        nc.sync.dma_start(
                            Wab[:, c * 512 : (c + 1) * 512],
                            ap_Wattn[128 * c : 128 * (c + 1), 512 * blk : 512 * (blk + 1)],
                        )
                    for g in range(G):
                        psp = psP.tile([128, 512], F32, tag="psp")
                        for c in range(HC):
                            nc.tensor.matmul(
                                psp[:],
                                Ag[:, (g * HC + c) * 128 : (g * HC + c) * 128 + 128],
                                Wab[:, c * 512 : (c + 1) * 512],
                                start=(c == 0),
                                stop=(c == HC - 1),
                            )
                        nc.vector.tensor_copy(
                            P_sb[:, g * H4 + 512 * blk : g * H4 + 512 * (blk + 1)],
                            psp[:],
                        )

            # ---------------------- phase C: recurrence ----------------------
            with tc.tile_pool(name="wrk", bufs=2) as wrk, \
                 tc.tile_pool(name="psc", bufs=2, space="PSUM") as psc_pool, \
                 tc.tile_pool(name="pwx", bufs=1, space="PSUM") as pwx_pool, \
                 tc.tile_pool(name="pstr", bufs=1, space="PSUM") as pstr_pool, \
                 tc.tile_pool(name="paT", bufs=1, space="PSUM") as paT_pool:
                q4 = lambda ap: ap.rearrange("p (q c) -> p q c", q=4)
                for t in range(t_steps):
                    # prefetched xproj slices for this step
                    xpt = [wrk.tile([128, 512], BF16, tag=f"xpt{r}", name=f"xpt{r}_{t}") for r in range(2)]
                    for r in range(2):
                        nc.sync.dma_start(xpt[r][:], xps[t, r])

                    # -- scores: per-group (8-sample) products against Ag with
                    # 4-way col tiling, group-local diag extract, softmax
                    pscg = psc_pool.tile([128, 128], F32, tag="psc")
                    for c in range(HC):
                        for g in range(G):
                            nc.tensor.matmul(
                                pscg[32 * g : 32 * (g + 1), :],
                                uTh[:, c * 32 : (c + 1) * 32],
                                Ag[:, (g * HC + c) * 128 : (g * HC + c + 1) * 128],
                                start=(c == 0),
                                stop=(c == HC - 1),
                                skip_group_check=True,
                                tile_position=(0, 32 * g),
                            )
                    scm = wrk.tile([128, 128], F32, tag="scm")
                    nc.vector.tensor_mul(scm[:], pscg[:], Mdiag8[:])
                    scores = wrk.tile([128, 16], F32, tag="scores")
                    nc.vector.tensor_reduce(
                        scores[:],
                        scm[:].rearrange("p (k n) -> p k n", k=16),
                        axis=AX.X,
                        op=ALU.add,
                    )
                    nmx = wrk.tile([128, 1], F32, tag="nmx")
                    nc.vector.tensor_reduce(
                        nmx[:], scores[:], axis=AX.X, op=ALU.max, negate=True
                    )
                    # softmax via the sigmoid table (keeps every ACT op in the
                    # sigmoid_and_others set -> one table load for the kernel):
                    # y = sigmoid(s - m) in (0, 0.5], e^(s-m) = y / (1 - y)
                    ysig = wrk.tile([128, 16], F32, tag="ysig")
                    nc.scalar.activation(
                        ysig[:], scores[:], AF.Sigmoid, bias=nmx[:], scale=1.0
                    )
                    omy = wrk.tile([128, 16], F32, tag="omy")
                    nc.vector.tensor_scalar(
                        omy[:], ysig[:], -1.0, 1.0, ALU.mult, ALU.add
                    )
                    romy = wrk.tile([128, 16], F32, tag="romy")
                    nc.vector.reciprocal(romy[:], omy[:])
                    ex = wrk.tile([128, 16], F32, tag="ex")
                    esum = wrk.tile([128, 1], F32, tag="esum")
                    nc.vector.scalar_tensor_tensor(
                        ex[:], ysig[:], 1.0, romy[:], ALU.mult, ALU.mult,
                        accum_out=esum[:],
                    )
                    rcp = wrk.tile([128, 1], F32, tag="rcp")
                    nc.vector.reciprocal(rcp[:], esum[:])
                    w2 = wrk.tile([128, 16], BF16, tag="w2")
                    nc.vector.tensor_scalar_mul(w2[:], ex[:], rcp[:])
                    # compact the (g, m)-partition w to wT[k, n32] on PE
                    wTps = pwx_pool.tile([16, 32], F32, tag="wTps")
                    nc.tensor.matmul(wTps[:], w2[:], selT[:], start=True, stop=True)
                    wT = wrk.tile([16, 32], BF16, tag="wT")
                    nc.vector.tensor_copy(wT[:], wTps[:])
                    wrep = wrk.tile([16, 128], BF16, tag="wrep")
                    for rep in range(4):
                        nc.vector.tensor_copy(
                            wrep[:, 32 * rep : 32 * (rep + 1)], wT[:]
                        )
                    pwx = pwx_pool.tile([128, 128], F32, tag="pwx")
                    nc.tensor.matmul(pwx[:], E16[:], wrep[:], start=True, stop=True)
                    masked = wrk.tile([128, 512], BF16, tag="masked")
                    for g in range(G):
                        nc.vector.tensor_mul(
                            masked[:, g * 128 : (g + 1) * 128],
                            pwx[:],
                            M32R[:, g * 128 : (g + 1) * 128],
                        )

                    # -- gates: h @ Wh + sum_k w_k P_k into 2 column-tiled strips
                    strips = [
                        pstr_pool.tile([128, 512], F32, tag=f"strip{r}",
                                       name=f"strip{r}_{t}")
                        for r in range(2)
                    ]
                    for c in range(HC):
                        for r in range(2):
                            for j in range(4):
                                nc.tensor.matmul(
                                    strips[r][32 * j : 32 * (j + 1), :],
                                    uTh[:, c * 32 : (c + 1) * 32],
                                    Wh_sb[:, c * H4 + j * 1024 + r * 512 : c * H4 + j * 1024 + r * 512 + 512],
                                    start=(c == 0),
                                    stop=False,
                                    skip_group_check=True,
                                    tile_position=(0, 32 * j),
                                )
                    for r in range(2):
                        for g in range(G):
                            for j in range(4):
                                nc.tensor.matmul(
                                    strips[r][32 * j : 32 * (j + 1), :],
                                    masked[:, g * 128 + 32 * j : g * 128 + 32 * (j + 1)],
                                    P_sb[:, g * H4 + j * 1024 + r * 512 : g * H4 + j * 1024 + r * 512 + 512],
                                    start=False,
                                    stop=(g == G - 1),
                                    skip_group_check=True,
                                    tile_position=(0, 32 * j),
                                )

                    # -- transpose strips, add xproj, activations, cell update
                    for r in range(2):
                        sg = wrk.tile([128, 512], BF16, tag=f"sg{r}")
                        nc.scalar.copy(sg[:], strips[r][:])
                        pat = paT_pool.tile([128, 512], BF16, tag=f"pat{r}")
                        for q in range(4):
                            nc.tensor.matmul(
                                pat[:, 128 * q : 128 * (q + 1)],
                                sg[:, 128 * q : 128 * (q + 1)],
                                eye[:],
                                is_transpose=True,
                                start=(q == 0),
                                stop=(q == 3),
                            )
                        ssum = wrk.tile([128, 512], BF16, tag=f"ssum{r}")
                        nc.vector.tensor_add(ssum[:], pat[:], xpt[r][:])
                        act = wrk.tile([128, 512], F32, tag=f"act{r}")
                        nc.scalar.activation(
                            q4(act[:])[:, :, 0:96], q4(ssum[:])[:, :, 0:96], AF.Sigmoid
                        )
                        nc.scalar.activation(
                            q4(act[:])[:, :, 96:128], q4(ssum[:])[:, :, 96:128], AF.Tanh
                        )
                        i_v = q4(act[:])[:, :, 0:32]
                        f_v = q4(act[:])[:, :, 32:64]
                        o_v = q4(act[:])[:, :, 64:96]
                        g_v = q4(act[:])[:, :, 96:128]
                        cview = cT[:, 128 * r : 128 * (r + 1)].rearrange(
                            "p (q n) -> p q n", q=4
                        )
                        ig = wrk.tile([128, 128], F32, tag=f"ig{r}")
                        nc.vector.tensor_mul(q4(ig[:]), i_v, g_v)
                        fc = wrk.tile([128, 128], F32, tag=f"fc{r}")
                        nc.vector.tensor_mul(q4(fc[:]), f_v, cview)
                        nc.vector.tensor_add(
                            cview, q4(ig[:]), q4(fc[:])
                        )
                        tch = wrk.tile([128, 128], F32, tag=f"tch{r}")
                        nc.scalar.activation(
                            tch[:], cT[:, 128 * r : 128 * (r + 1)], AF.Tanh
                        )
                        h32 = wrk.tile([128, 128], F32, tag=f"h32{r}")
                        nc.vector.tensor_mul(
                            h32[:].rearrange("p (q n) -> p q n", q=4),
                            o_v,
                            tch[:].rearrange("p (q n) -> p q n", q=4),
                        )
                        # write h into uTh (bf16) for step t+1
                        nc.vector.tensor_copy(
                            uTh[:, 128 * r : 128 * (r + 1)], h32[:]
                        )
                        nc.sync.dma_start(
                            outT[t, 512 * r : 512 * (r + 1), :].rearrange(
                                "(q p) n -> p q n", p=128
                            ),
                            h32[:].rearrange("p (q n) -> p q n", q=4),
                        )
    nc.compile()
    return nc


def _prep_shards(inputs):
    x = np.asarray(inputs["x"], np.float32)
    A = np.asarray(inputs["A"], np.float32)
    Wx = np.asarray(inputs["Wx"], np.float32)
    Wh = np.asarray(inputs["Wh"], np.float32)
    Wattn = np.asarray(inputs["Wattn"], np.float32)
    b = np.asarray(inputs["b"], np.float32)

    Wx_bf = np.ascontiguousarray(Wx.astype(BF))
    Wh_bf = np.ascontiguousarray(Wh.astype(BF))
    Wa_bf = np.ascontiguousarray(Wattn.astype(BF))
    bT = np.ascontiguousarray(b.reshape(32, 128).T.astype(np.float32))

    in_maps = []
    for i in range(NCORES):
        ns = slice(NL * i, NL * (i + 1))
        xT = x[ns].transpose(2, 1, 0).reshape(D, T * NL)
        Asc = A[ns].reshape(NL, H, 16).transpose(1, 2, 0).reshape(H, 512)
        in_maps.append(
            {
                "xT": np.ascontiguousarray(xT.astype(BF)),
                "Asc": np.ascontiguousarray(Asc.astype(BF)),
                "Asc32": np.ascontiguousarray(Asc.astype(np.float32)),
                "Wx": Wx_bf,
                "Wh": Wh_bf,
                "Wattn": Wa_bf,
                "bT": bT,
            }
        )
    return in_maps


def _get_nc():
    global _built
    if _built is None:
        _built = _build_nc()
    return _built


def _run(inputs, **kwargs):
    nc = _get_nc()
    in_maps = _prep_shards(inputs)
    res = bass_utils.run_bass_kernel_spmd(
        nc, in_maps, core_ids=list(range(NCORES)), **kwargs
    )
    out = np.empty((N, T, H), np.float32)
    for i in range(NCORES):
        out[NL * i : NL * (i + 1)] = res.results[i]["outT"].transpose(2, 0, 1)
    return out, res


def kernel(**inputs):
    out, _ = _run(inputs)
    return out



# revision 28
# speedup vs baseline: 1.2099x; 1.0260x over previous
"""Trainium2 Bass kernel for nn_CaptioningRNN (attention LSTM over T=64).

Data-parallel over the batch: N=256 samples split across 8 NeuronCores
(32 samples/core), weights replicated, no collectives.

Per-core algorithm (all matmuls bf16 on the TensorEngine, state in f32):
  1. xproj phase: xpT = (x @ Wx + b) computed transposed via Wx-stationary
     matmuls, stored to a DRAM scratch laid out so the per-step slice loads
     as a clean [128, 512] tile.
  2. P phase: P[n, k, :] = A[n, :, k] @ Wattn precomputed once (the
     attention context contribution to the gates becomes a w-weighted sum
     of P rows, replacing a per-step [32,1024]@[1024,4096] matmul).
     h0 = c0 = mean_k(A) computed on device from an f32 copy of A.
  3. Recurrence (64 steps):
     - scores via hT-chunk matmuls against a permuted A (cross-sample
       products in PSUM, diagonal extracted with a mask+reduce on DVE)
     - softmax on [32,16] (ACT exp with fused row-sum)
     - w transposed (DVE 32x32 stream transpose) and expanded to the
       (k, n_g)-partition block-diagonal layout via a one-hot matmul + mask
     - gates = h @ Wh + sum_k w_k P_k accumulated into two PSUM strips
       using 4-way tensor-engine column tiling (beats the M=32 small-batch
       penalty ~4x)
     - strips transposed on PE; cell math done in h-on-partition space so
       i/f/o/g land on identical lanes (no cross-partition ops needed)
  4. Output written transposed [t, h, n]; host reassembles to (N, T, H).
"""

from contextlib import ExitStack

import numpy as np
import ml_dtypes

import concourse.bacc as bacc
import concourse.mybir as mybir
from concourse import bass_utils
from concourse.tile import TileContext

F32, BF16 = mybir.dt.float32, mybir.dt.bfloat16
AF = mybir.ActivationFunctionType
ALU = mybir.AluOpType
AX = mybir.AxisListType
BF = ml_dtypes.bfloat16

N, T, D, H = 256, 64, 1024, 1024
NCORES = 8
NL = N // NCORES          # 32 samples per core
HC = 8                    # 128-row chunks of D/H
G, GS = 4, 8              # sample groups of 8 (for the (k, n_g) 128-partition layout)
H4 = 4 * H                # 4096 gate columns

_built = None


def _consts():
    # E16[k', 8k + n] = (k' == k): one-hot expansion of wT rows onto the
    # (k-major, n_g-minor) 128-partition layout.
    e16 = np.zeros((16, 128), dtype=BF)
    for k in range(16):
        e16[k, 8 * k : 8 * k + 8] = 1
    # M32R[p, 128 g + 32 rep + m] = (m % 8 == p % 8) & (m // 8 == g):
    # block-diagonal mask producing masked_g = w[m, k(p)] only for group-g
    # samples, replicated 4x for the column-tiled matmuls.
    p = np.arange(128)[:, None]
    m = np.arange(32)[None, :]
    m32r = np.zeros((128, 512), dtype=BF)
    for g in range(4):
        blk = ((m % 8 == p % 8) & (m // 8 == g)).astype(BF)
        for rep in range(4):
            m32r[:, 128 * g + 32 * rep : 128 * g + 32 * rep + 32] = blk
    # Mdiag8[32 g + m, 8 k + n] = (m == 8 g + n) / 32: extracts the
    # group-local diagonal of the score products (stationary = all 32
    # samples, moving = group-g A columns) and applies the 1/sqrt(H) scale.
    md8 = np.zeros((128, 128), dtype=np.float32)
    for g in range(4):
        for n in range(8):
            for k in range(16):
                md8[32 * g + 8 * g + n, 8 * k + n] = 1.0 / 32.0
    # selT[32 g + (8 g + n), 8 g + n] = 1: compacts the block-diagonal w
    # layout to wT[k, n] via a single PE matmul (stationary = w2).
    sel = np.zeros((128, 32), dtype=BF)
    for g in range(4):
        for n in range(8):
            sel[32 * g + 8 * g + n, 8 * g + n] = 1
    return e16, m32r, md8, sel


def _build_nc(t_steps=T):
    nc = bacc.Bacc(trn_type="TRN2", target_bir_lowering=False, debug=False)

    ap_xT = nc.dram_tensor("xT", [D, T * NL], BF16, kind="ExternalInput").ap()
    ap_Asc = nc.dram_tensor("Asc", [H, 512], BF16, kind="ExternalInput").ap()
    ap_Asc32 = nc.dram_tensor("Asc32", [H, 512], F32, kind="ExternalInput").ap()
    ap_Wx = nc.dram_tensor("Wx", [D, H4], BF16, kind="ExternalInput").ap()
    ap_Wh = nc.dram_tensor("Wh", [H, H4], BF16, kind="ExternalInput").ap()
    ap_Wattn = nc.dram_tensor("Wattn", [H, H4], BF16, kind="ExternalInput").ap()
    ap_bT = nc.dram_tensor("bT", [128, 32], F32, kind="ExternalInput").ap()
    outT = nc.dram_tensor("outT", [T, H, NL], F32, kind="ExternalOutput").ap()
    # xps[t, r, p, q, j, n] = xproj[t][n, j*1024 + r*512 + q*128 + p]
    xps = nc.dram_tensor("xps", [T, 2, 128, 4, 4, NL], BF16, kind="Internal").ap()

    e16_np, m32r_np, md8_np, sel_np = _consts()
    eye_d = nc.inline_tensor(np.eye(128, dtype=BF), "c_eye")
    e16_d = nc.inline_tensor(e16_np, "c_e16")
    m32r_d = nc.inline_tensor(m32r_np, "c_m32r")
    md8_d = nc.inline_tensor(md8_np, "c_mdiag8")
    sel_d = nc.inline_tensor(sel_np, "c_selT")

    with TileContext(nc) as tc:
        with tc.tile_pool(name="pers", bufs=1) as pers:
            Wh_sb = pers.tile([128, HC * H4], BF16, tag="Wh")
            Asc_sb = pers.tile([128, HC * 512], BF16, tag="Asc")
            P_sb = pers.tile([128, G * H4], BF16, tag="P")
            uTh = pers.tile([128, HC * 32], BF16, tag="uTh")
            cT = pers.tile([128, 256], F32, tag="cT")
            eye = pers.tile([128, 128], BF16, tag="eye")
            E16 = pers.tile([16, 128], BF16, tag="E16")
            M32R = pers.tile([128, 512], BF16, tag="M32R")
            Mdiag8 = pers.tile([128, 128], F32, tag="Mdiag8")
            selT = pers.tile([128, 32], BF16, tag="selT")
            b_sb = pers.tile([128, 32], F32, tag="bT")
            Ag = pers.tile([128, G * HC * 128], BF16, tag="Ag")

            nc.sync.dma_start(eye[:], eye_d.ap()[:])
            nc.sync.dma_start(E16[:], e16_d.ap()[:])
            nc.sync.dma_start(M32R[:], m32r_d.ap()[:])
            nc.sync.dma_start(Mdiag8[:], md8_d.ap()[:])
            nc.sync.dma_start(selT[:], sel_d.ap()[:])
            nc.sync.dma_start(b_sb[:], ap_bT[:])
            for c in range(HC):
                nc.sync.dma_start(
                    Wh_sb[:, c * H4 : (c + 1) * H4], ap_Wh[128 * c : 128 * (c + 1), :]
                )
                nc.sync.dma_start(
                    Asc_sb[:, c * 512 : (c + 1) * 512],
                    ap_Asc[128 * c : 128 * (c + 1), :],
                )

            # ---------------- phase A: xproj -> DRAM scratch ----------------
            # Pools stay open through the recurrence so t4>=1 chunks can be
            # interleaved between steps (fills PE-idle gaps, keeps HAM warm).
            _ax = ExitStack()
            phx1 = _ax.enter_context(tc.tile_pool(name="phx1", bufs=1))
            phx = _ax.enter_context(tc.tile_pool(name="phx", bufs=3))
            psX = _ax.enter_context(tc.tile_pool(name="psX", bufs=2, space="PSUM"))
            xT_sb = phx1.tile([128, HC * T * NL], BF16, tag="xTsb")
            for c in range(HC):
                nc.sync.dma_start(
                    xT_sb[:, c * T * NL : (c + 1) * T * NL],
                    ap_xT[128 * c : 128 * (c + 1), :],
                )

            def xproj_chunk(W, t4):
                j, r, q = W // 8, (W % 8) // 4, W % 4
                Wxb = phx.tile(
                    [128, HC * 128], BF16, tag="Wxb", name=f"Wxb_{W}_{t4}"
                )
                for c in range(HC):
                    nc.sync.dma_start(
                        Wxb[:, c * 128 : (c + 1) * 128],
                        ap_Wx[128 * c : 128 * (c + 1), 128 * W : 128 * (W + 1)],
                    )
                psx = psX.tile([128, 512], F32, tag="psx", name=f"psx_{W}_{t4}")
                for c in range(HC):
                    nc.tensor.matmul(
                        psx[:],
                        Wxb[:, c * 128 : (c + 1) * 128],
                        xT_sb[:, c * T * NL + 512 * t4 : c * T * NL + 512 * (t4 + 1)],
                        start=(c == 0),
                        stop=(c == HC - 1),
                    )
                sxp = phx.tile([128, 512], BF16, tag="sxp", name=f"sxp_{W}_{t4}")
                nc.vector.tensor_scalar_add(sxp[:], psx[:], b_sb[:, W : W + 1])
                nc.sync.dma_start(
                    xps[16 * t4 : 16 * (t4 + 1), r, :, q, j, :].transpose(
                        [1, 0, 2]
                    ),
                    sxp[:].rearrange("p (t n) -> p t n", t=16),
                )

            for W in range(32):
                xproj_chunk(W, 0)

            # ------------- phase B: P precompute + h0/c0 init -------------
            with tc.tile_pool(name="php1", bufs=1) as php1, \
                 tc.tile_pool(name="php", bufs=3) as php, \
                 tc.tile_pool(name="psP", bufs=2, space="PSUM") as psP:
                A32 = php1.tile([128, HC * 512], F32, tag="A32")
                for c in range(HC):
                    nc.sync.dma_start(
                        A32[:, c * 512 : (c + 1) * 512],
                        ap_Asc32[128 * c : 128 * (c + 1), :],
                    )
                for c in range(HC):
                    h0s = php.tile([128, 32], F32, tag="h0s")
                    nc.vector.tensor_reduce(
                        h0s[:],
                        A32[:, c * 512 : (c + 1) * 512].rearrange(
                            "p (k n) -> p n k", k=16
                        ),
                        axis=AX.X,
                        op=ALU.add,
                    )
                    nc.vector.tensor_scalar_mul(
                        cT[:, 32 * c : 32 * (c + 1)], h0s[:], 1.0 / 16.0
                    )
                    nc.vector.tensor_copy(
                        uTh[:, 32 * c : 32 * (c + 1)],
                        cT[:, 32 * c : 32 * (c + 1)],
                    )
                # contiguous staging of the group-selected A columns so the
                # matmul stationary operand has a single free dim
                for g in range(G):
                    for c in range(HC):
                        nc.vector.tensor_copy(
                            Ag[:, (g * HC + c) * 128 : (g * HC + c) * 128 + 128],
                            Asc_sb[:, c * 512 : (c + 1) * 512].rearrange(
                                "p (k n) -> p k n", k=16
                            )[:, :, GS * g : GS * (g + 1)],
                        )
                for blk in range(8):
                    Wab = php.tile([128, HC * 512], BF16, tag="Wab")
                    for c in range(HC):
                        nc.sync.dma_start(
                            Wab[:, c * 512 : (c + 1) * 512],
                            ap_Wattn[128 * c : 128 * (c + 1), 512 * blk : 512 * (blk + 1)],
                        )
                    for g in range(G):
                        psp = psP.tile([128, 512], F32, tag="psp")
                        for c in range(HC):
                            nc.tensor.matmul(
                                psp[:],
                                Ag[:, (g * HC + c) * 128 : (g * HC + c) * 128 + 128],
                                Wab[:, c * 512 : (c + 1) * 512],
                                start=(c == 0),
                                stop=(c == HC - 1),
                            )
                        nc.vector.tensor_copy(
                            P_sb[:, g * H4 + 512 * blk : g * H4 + 512 * (blk + 1)],
                            psp[:],
                        )

            # ---------------------- phase C: recurrence ----------------------
            with tc.tile_pool(name="wrk", bufs=2) as wrk, \
                 tc.tile_pool(name="psc", bufs=1, space="PSUM") as psc_pool, \
                 tc.tile_pool(name="pwx", bufs=1, space="PSUM") as pwx_pool, \
                 tc.tile_pool(name="pstr", bufs=1, space="PSUM") as pstr_pool, \
                 tc.tile_pool(name="paT", bufs=1, space="PSUM") as paT_pool:
                q4 = lambda ap: ap.rearrange("p (q c) -> p q c", q=4)
                chunks = [
                    (W, t4)
                    for t4 in (1, 2, 3)
                    if 16 * t4 < t_steps
                    for W in range(32)
                ]
                ci = 0
                for t in range(t_steps):
                    # prefetched xproj slices for this step
                    xpt = [wrk.tile([128, 512], BF16, tag=f"xpt{r}", name=f"xpt{r}_{t}") for r in range(2)]
                    for r in range(2):
                        nc.sync.dma_start(xpt[r][:], xps[t, r])

                    # -- scores: per-group (8-sample) products against Ag with
                    # 4-way col tiling, group-local diag extract, softmax
                    pscg = psc_pool.tile([128, 128], F32, tag="psc")
                    for c in range(HC):
                        for g in range(G):
                            nc.tensor.matmul(
                                pscg[32 * g : 32 * (g + 1), :],
                                uTh[:, c * 32 : (c + 1) * 32],
                                Ag[:, (g * HC + c) * 128 : (g * HC + c + 1) * 128],
                                start=(c == 0),
                                stop=(c == HC - 1),
                                skip_group_check=True,
                                tile_position=(0, 32 * g),
                            )
                    scm = wrk.tile([128, 128], F32, tag="scm")
                    nc.vector.tensor_mul(scm[:], pscg[:], Mdiag8[:])
                    scores = wrk.tile([128, 16], F32, tag="scores")
                    nc.vector.tensor_reduce(
                        scores[:],
                        scm[:].rearrange("p (k n) -> p k n", k=16),
                        axis=AX.X,
                        op=ALU.add,
                    )
                    nmx = wrk.tile([128, 1], F32, tag="nmx")
                    nc.vector.tensor_reduce(
                        nmx[:], scores[:], axis=AX.X, op=ALU.max, negate=True
                    )
                    # softmax via the sigmoid table (keeps every ACT op in the
                    # sigmoid_and_others set -> one table load for the kernel):
                    # y = sigmoid(s - m) in (0, 0.5], e^(s-m) = y / (1 - y)
                    ysig = wrk.tile([128, 16], F32, tag="ysig")
                    nc.scalar.activation(
                        ysig[:], scores[:], AF.Sigmoid, bias=nmx[:], scale=1.0
                    )
                    omy = wrk.tile([128, 16], F32, tag="omy")
                    nc.vector.tensor_scalar(
                        omy[:], ysig[:], -1.0, 1.0, ALU.mult, ALU.add
                    )
                    romy = wrk.tile([128, 16], F32, tag="romy")
                    nc.vector.reciprocal(romy[:], omy[:])
                    ex = wrk.tile([128, 16], F32, tag="ex")
                    esum = wrk.tile([128, 1], F32, tag="esum")
                    nc.vector.scalar_tensor_tensor(
                        ex[:], ysig[:], 1.0, romy[:], ALU.mult, ALU.mult,
                        accum_out=esum[:],
                    )
                    rcp = wrk.tile([128, 1], F32, tag="rcp")
                    nc.vector.reciprocal(rcp[:], esum[:])
                    w2 = wrk.tile([128, 16], BF16, tag="w2")
                    nc.vector.tensor_scalar_mul(w2[:], ex[:], rcp[:])
                    # compact the (g, m)-partition w to wT[k, n32] on PE
                    wTps = pwx_pool.tile([16, 32], F32, tag="wTps")
                    nc.tensor.matmul(wTps[:], w2[:], selT[:], start=True, stop=True)
                    wT = wrk.tile([16, 32], BF16, tag="wT")
                    nc.vector.tensor_copy(wT[:], wTps[:])
                    wrep = wrk.tile([16, 128], BF16, tag="wrep")
                    for rep in range(4):
                        nc.vector.tensor_copy(
                            wrep[:, 32 * rep : 32 * (rep + 1)], wT[:]
                        )
                    pwx = pwx_pool.tile([128, 128], F32, tag="pwx")
                    nc.tensor.matmul(pwx[:], E16[:], wrep[:], start=True, stop=True)
                    masked = wrk.tile([128, 512], BF16, tag="masked")
                    for g in range(G):
                        nc.vector.tensor_mul(
                            masked[:, g * 128 : (g + 1) * 128],
                            pwx[:],
                            M32R[:, g * 128 : (g + 1) * 128],
                        )

                    # -- gates: h @ Wh + sum_k w_k P_k into 2 column-tiled strips
                    strips = [
                        pstr_pool.tile([128, 512], F32, tag=f"strip{r}",
                                       name=f"strip{r}_{t}")
                        for r in range(2)
                    ]
                    for c in range(HC):
                        for r in range(2):
                            for j in range(4):
                                nc.tensor.matmul(
                                    strips[r][32 * j : 32 * (j + 1), :],
                                    uTh[:, c * 32 : (c + 1) * 32],
                                    Wh_sb[:, c * H4 + j * 1024 + r * 512 : c * H4 + j * 1024 + r * 512 + 512],
                                    start=(c == 0),
                                    stop=False,
                                    skip_group_check=True,
                                    tile_position=(0, 32 * j),
                                )
                    for r in range(2):
                        for g in range(G):
                            for j in range(4):
                                nc.tensor.matmul(
                                    strips[r][32 * j : 32 * (j + 1), :],
                                    masked[:, g * 128 + 32 * j : g * 128 + 32 * (j + 1)],
                                    P_sb[:, g * H4 + j * 1024 + r * 512 : g * H4 + j * 1024 + r * 512 + 512],
                                    start=False,
                                    stop=(g == G - 1),
                                    skip_group_check=True,
                                    tile_position=(0, 32 * j),
                                )

                    # -- transpose strips, add xproj, activations, cell update
                    for r in range(2):
                        sg = wrk.tile([128, 512], BF16, tag=f"sg{r}")
                        nc.scalar.copy(sg[:], strips[r][:])
                        pat = paT_pool.tile(
                            [128, 512], BF16, tag="pat", name=f"pat{r}_{t}"
                        )
                        for q in range(4):
                            nc.tensor.matmul(
                                pat[:, 128 * q : 128 * (q + 1)],
                                sg[:, 128 * q : 128 * (q + 1)],
                                eye[:],
                                is_transpose=True,
                                start=(q == 0),
                                stop=(q == 3),
                            )
                        ssum = wrk.tile([128, 512], BF16, tag=f"ssum{r}")
                        nc.vector.tensor_add(ssum[:], pat[:], xpt[r][:])
                        act = wrk.tile([128, 512], F32, tag=f"act{r}")
                        nc.scalar.activation(
                            q4(act[:])[:, :, 0:96], q4(ssum[:])[:, :, 0:96], AF.Sigmoid
                        )
                        nc.scalar.activation(
                            q4(act[:])[:, :, 96:128], q4(ssum[:])[:, :, 96:128], AF.Tanh
                        )
                        i_v = q4(act[:])[:, :, 0:32]
                        f_v = q4(act[:])[:, :, 32:64]
                        o_v = q4(act[:])[:, :, 64:96]
                        g_v = q4(act[:])[:, :, 96:128]
                        cview = cT[:, 128 * r : 128 * (r + 1)].rearrange(
                            "p (q n) -> p q n", q=4
                        )
                        ig = wrk.tile([128, 128], F32, tag=f"ig{r}")
                        nc.vector.tensor_mul(q4(ig[:]), i_v, g_v)
                        fc = wrk.tile([128, 128], F32, tag=f"fc{r}")
                        nc.vector.tensor_mul(q4(fc[:]), f_v, cview)
                        nc.vector.tensor_add(
                            cview, q4(ig[:]), q4(fc[:])
                        )
                        tch = wrk.tile([128, 128], F32, tag=f"tch{r}")
                        nc.scalar.activation(
                            tch[:], cT[:, 128 * r : 128 * (r + 1)], AF.Tanh
                        )
                        h32 = wrk.tile([128, 128], F32, tag=f"h32{r}")
                        nc.vector.tensor_mul(
                            h32[:].rearrange("p (q n) -> p q n", q=4),
                            o_v,
                            tch[:].rearrange("p (q n) -> p q n", q=4),
                        )
                        # write h into uTh (bf16) for step t+1
                        nc.vector.tensor_copy(
                            uTh[:, 128 * r : 128 * (r + 1)], h32[:]
                        )
                        nc.sync.dma_start(
                            outT[t, 512 * r : 512 * (r + 1), :].rearrange(
                                "(q p) n -> p q n", p=128
                            ),
                            h32[:].rearrange("p (q n) -> p q n", q=4),
                        )

                    # interleave deferred xproj chunks (t4 >= 1) into the
                    # recurrence to fill PE-idle gaps and keep HAM warm
                    want = min(len(chunks), int((t + 1) * 2.4) + 1)
                    while ci < want:
                        xproj_chunk(*chunks[ci])
                        ci += 1
                while ci < len(chunks):
                    xproj_chunk(*chunks[ci])
                    ci += 1
            _ax.close()
    nc.compile()
    return nc


def _prep_shards(inputs):
    x = np.asarray(inputs["x"], np.float32)
    A = np.asarray(inputs["A"], np.float32)
    Wx = np.asarray(inputs["Wx"], np.float32)
    Wh = np.asarray(inputs["Wh"], np.float32)
    Wattn = np.asarray(inputs["Wattn"], np.float32)
    b = np.asarray(inputs["b"], np.float32)

    Wx_bf = np.ascontiguousarray(Wx.astype(BF))
    Wh_bf = np.ascontiguousarray(Wh.astype(BF))
    Wa_bf = np.ascontiguousarray(Wattn.astype(BF))
    bT = np.ascontiguousarray(b.reshape(32, 128).T.astype(np.float32))

    in_maps = []
    for i in range(NCORES):
        ns = slice(NL * i, NL * (i + 1))
        xT = x[ns].transpose(2, 1, 0).reshape(D, T * NL)
        Asc = A[ns].reshape(NL, H, 16).transpose(1, 2, 0).reshape(H, 512)
        in_maps.append(
            {
                "xT": np.ascontiguousarray(xT.astype(BF)),
                "Asc": np.ascontiguousarray(Asc.astype(BF)),
                "Asc32": np.ascontiguousarray(Asc.astype(np.float32)),
                "Wx": Wx_bf,
                "Wh": Wh_bf,
                "Wattn": Wa_bf,
                "bT": bT,
            }
        )
    return in_maps


def _get_nc():
    global _built
    if _built is None:
        _built = _build_nc()
    return _built


def _run(inputs, **kwargs):
    nc = _get_nc()
    in_maps = _prep_shards(inputs)
    res = bass_utils.run_bass_kernel_spmd(
        nc, in_maps, core_ids=list(range(NCORES)), **kwargs
    )
    out = np.empty((N, T, H), np.float32)
    for i in range(NCORES):
        out[NL * i : NL * (i + 1)] = res.results[i]["outT"].transpose(2, 0, 1)
    return out, res


def kernel(**inputs):
    out, _ = _run(inputs)
    return out



# revision 44
# speedup vs baseline: 1.4480x; 1.1967x over previous
"""Trainium2 Bass kernel for nn_CaptioningRNN (attention LSTM over T=64).

Data-parallel over the batch: N=256 samples split across 8 NeuronCores
(32 samples/core), weights replicated, no collectives.

Per-core algorithm (all matmuls bf16 on the TensorEngine, state in f32):
  1. xproj phase: xpT = (x @ Wx + b) computed transposed via Wx-stationary
     matmuls, stored to a DRAM scratch laid out so the per-step slice loads
     as a clean [128, 512] tile.
  2. P phase: P[n, k, :] = A[n, :, k] @ Wattn precomputed once (the
     attention context contribution to the gates becomes a w-weighted sum
     of P rows, replacing a per-step [32,1024]@[1024,4096] matmul).
     h0 = c0 = mean_k(A) computed on device from an f32 copy of A.
  3. Recurrence (64 steps):
     - scores via hT-chunk matmuls against a permuted A (cross-sample
       products in PSUM, diagonal extracted with a mask+reduce on DVE)
     - softmax on [32,16] (ACT exp with fused row-sum)
     - w transposed (DVE 32x32 stream transpose) and expanded to the
       (k, n_g)-partition block-diagonal layout via a one-hot matmul + mask
     - gates = h @ Wh + sum_k w_k P_k accumulated into two PSUM strips
       using 4-way tensor-engine column tiling (beats the M=32 small-batch
       penalty ~4x)
     - strips transposed on PE; cell math done in h-on-partition space so
       i/f/o/g land on identical lanes (no cross-partition ops needed)
  4. Output written transposed [t, h, n]; host reassembles to (N, T, H).
"""

from contextlib import ExitStack

import numpy as np
import ml_dtypes

import concourse.bacc as bacc
import concourse.mybir as mybir
from concourse import bass_utils
from concourse.tile import TileContext

F32, BF16 = mybir.dt.float32, mybir.dt.bfloat16
AF = mybir.ActivationFunctionType
ALU = mybir.AluOpType
AX = mybir.AxisListType
BF = ml_dtypes.bfloat16

N, T, D, H = 256, 64, 1024, 1024
NCORES = 8
NL = N // NCORES          # 32 samples per core
HC = 8                    # 128-row chunks of D/H
G, GS = 4, 8              # sample groups of 8 (for the (k, n_g) 128-partition layout)
H4 = 4 * H                # 4096 gate columns

_built = None


def _consts():
    # E16[k', 8k + n] = (k' == k): one-hot expansion of wT rows onto the
    # (k-major, n_g-minor) 128-partition layout.
    e16 = np.zeros((16, 128), dtype=BF)
    for k in range(16):
        e16[k, 8 * k : 8 * k + 8] = 1
    # M32R[p, 128 g + 32 rep + m] = (m % 8 == p % 8) & (m // 8 == g):
    # block-diagonal mask producing masked_g = w[m, k(p)] only for group-g
    # samples, replicated 4x for the column-tiled matmuls.
    p = np.arange(128)[:, None]
    m = np.arange(32)[None, :]
    m32r = np.zeros((128, 512), dtype=BF)
    for g in range(4):
        blk = ((m % 8 == p % 8) & (m // 8 == g)).astype(BF)
        for rep in range(4):
            m32r[:, 128 * g + 32 * rep : 128 * g + 32 * rep + 32] = blk
    # Mdiag8[32 g + m, 8 k + n] = (m == 8 g + n) / 32: extracts the
    # group-local diagonal of the score products (stationary = all 32
    # samples, moving = group-g A columns) and applies the 1/sqrt(H) scale.
    md8 = np.zeros((128, 128), dtype=np.float32)
    for g in range(4):
        for n in range(8):
            for k in range(16):
                md8[32 * g + 8 * g + n, 8 * k + n] = 1.0 / 32.0
    # selT[32 g + (8 g + n), 8 g + n] = 1: compacts the block-diagonal w
    # layout to wT[k, n] via a single PE matmul (stationary = w2).
    sel = np.zeros((128, 32), dtype=BF)
    for g in range(4):
        for n in range(8):
            sel[32 * g + 8 * g + n, 8 * g + n] = 1
    return e16, m32r, md8, sel


def _build_nc(t_steps=T):
    nc = bacc.Bacc(trn_type="TRN2", target_bir_lowering=False, debug=False)

    ap_xT = nc.dram_tensor("xT", [D, T * NL], BF16, kind="ExternalInput").ap()
    ap_Asc = nc.dram_tensor("Asc", [H, 512], BF16, kind="ExternalInput").ap()
    ap_Asc32 = nc.dram_tensor("Asc32", [H, 512], F32, kind="ExternalInput").ap()
    ap_Wx = nc.dram_tensor("Wx", [D, H4], BF16, kind="ExternalInput").ap()
    ap_Wh = nc.dram_tensor("Wh", [H, H4], BF16, kind="ExternalInput").ap()
    ap_Wattn = nc.dram_tensor("Wattn", [H, H4], BF16, kind="ExternalInput").ap()
    ap_bT = nc.dram_tensor("bT", [128, 32], F32, kind="ExternalInput").ap()
    outT = nc.dram_tensor("outT", [T, H, NL], F32, kind="ExternalOutput").ap()
    # xps[r, q, j, t, p, n] = xproj[t][n, j*1024 + r*512 + q*128 + p]
    # ((q, j) outermost so phase-A stores and per-step loads are both
    # contiguous 8 KiB (p, n) blocks per (q, j))
    xps = nc.dram_tensor("xps", [2, 4, 4, T, 128, NL], BF16, kind="Internal").ap()

    e16_np, m32r_np, md8_np, sel_np = _consts()
    eye_d = nc.inline_tensor(np.eye(128, dtype=BF), "c_eye")
    e16_d = nc.inline_tensor(e16_np, "c_e16")
    m32r_d = nc.inline_tensor(m32r_np, "c_m32r")
    md8_d = nc.inline_tensor(md8_np, "c_mdiag8")
    sel_d = nc.inline_tensor(sel_np, "c_selT")

    with TileContext(nc) as tc:
        with tc.tile_pool(name="pers", bufs=1) as pers:
            Wh_sb = pers.tile([128, HC * H4], BF16, tag="Wh")
            Asc_sb = pers.tile([128, HC * 512], BF16, tag="Asc")
            P_sb = pers.tile([128, G * H4], BF16, tag="P")
            uTh = pers.tile([128, HC * 32], BF16, tag="uTh")
            cT = pers.tile([128, 256], F32, tag="cT")
            eye = pers.tile([128, 128], BF16, tag="eye")
            E16 = pers.tile([16, 128], BF16, tag="E16")
            M32R = pers.tile([128, 512], BF16, tag="M32R")
            Mdiag8 = pers.tile([128, 128], F32, tag="Mdiag8")
            selT = pers.tile([128, 32], BF16, tag="selT")
            b_sb = pers.tile([128, 32], F32, tag="bT")
            Ag = pers.tile([128, G * HC * 128], BF16, tag="Ag")

            nc.sync.dma_start(eye[:], eye_d.ap()[:])
            nc.sync.dma_start(E16[:], e16_d.ap()[:])
            nc.sync.dma_start(M32R[:], m32r_d.ap()[:])
            nc.sync.dma_start(Mdiag8[:], md8_d.ap()[:])
            nc.sync.dma_start(selT[:], sel_d.ap()[:])
            nc.sync.dma_start(b_sb[:], ap_bT[:])
            nc.sync.dma_start(
                Wh_sb[:].rearrange("p (c x) -> p c x", c=HC),
                ap_Wh.rearrange("(c p) x -> p c x", p=128),
            )
            nc.sync.dma_start(
                Asc_sb[:].rearrange("p (c x) -> p c x", c=HC),
                ap_Asc.rearrange("(c p) x -> p c x", p=128),
            )

            # ---------------- phase A: xproj -> DRAM scratch ----------------
            # Pools stay open through the recurrence so t4>=1 chunks can be
            # interleaved between steps (fills PE-idle gaps, keeps HAM warm).
            _ax = ExitStack()
            phx1 = _ax.enter_context(tc.tile_pool(name="phx1", bufs=1))
            phx = _ax.enter_context(tc.tile_pool(name="phx", bufs=3))
            psX = _ax.enter_context(tc.tile_pool(name="psX", bufs=2, space="PSUM"))
            xT_sb = phx1.tile([128, HC * T * NL], BF16, tag="xTsb")
            nc.sync.dma_start(
                xT_sb[:].rearrange("p (c x) -> p c x", c=HC),
                ap_xT.rearrange("(c p) x -> p c x", p=128),
            )

            def xproj_chunk(W, t4):
                j, r, q = W // 8, (W % 8) // 4, W % 4
                Wxb = phx.tile(
                    [128, HC * 128], BF16, tag="Wxb", name=f"Wxb_{W}_{t4}"
                )
                nc.sync.dma_start(
                    Wxb[:].rearrange("p (c x) -> p c x", c=HC),
                    ap_Wx.rearrange("(c p) x -> p c x", p=128)[
                        :, :, 128 * W : 128 * (W + 1)
                    ],
                )
                psx = psX.tile([128, 512], F32, tag="psx", name=f"psx_{W}_{t4}")
                for c in range(HC):
                    nc.tensor.matmul(
                        psx[:],
                        Wxb[:, c * 128 : (c + 1) * 128],
                        xT_sb[:, c * T * NL + 512 * t4 : c * T * NL + 512 * (t4 + 1)],
                        start=(c == 0),
                        stop=(c == HC - 1),
                    )
                sxp = phx.tile([128, 512], BF16, tag="sxp", name=f"sxp_{W}_{t4}")
                nc.scalar.add(sxp[:], psx[:], b_sb[:, W : W + 1])
                nc.sync.dma_start(
                    xps[r, q, j, 16 * t4 : 16 * (t4 + 1)].transpose([1, 0, 2]),
                    sxp[:].rearrange("p (t n) -> p t n", t=16),
                )

            for W in range(32):
                xproj_chunk(W, 0)

            # ------------- phase B: P precompute + h0/c0 init -------------
            with tc.tile_pool(name="php1", bufs=1) as php1, \
                 tc.tile_pool(name="php", bufs=3) as php, \
                 tc.tile_pool(name="psP", bufs=2, space="PSUM") as psP:
                A32 = php1.tile([128, HC * 512], F32, tag="A32")
                nc.sync.dma_start(
                    A32[:].rearrange("p (c x) -> p c x", c=HC),
                    ap_Asc32.rearrange("(c p) x -> p c x", p=128),
                )
                for c in range(HC):
                    h0s = php.tile([128, 32], F32, tag="h0s")
                    nc.vector.tensor_reduce(
                        h0s[:],
                        A32[:, c * 512 : (c + 1) * 512].rearrange(
                            "p (k n) -> p n k", k=16
                        ),
                        axis=AX.X,
                        op=ALU.add,
                    )
                    nc.vector.tensor_scalar_mul(
                        cT[:, 32 * c : 32 * (c + 1)], h0s[:], 1.0 / 16.0
                    )
                    nc.vector.tensor_copy(
                        uTh[:, 32 * c : 32 * (c + 1)],
                        cT[:, 32 * c : 32 * (c + 1)],
                    )
                # contiguous staging of the group-selected A columns so the
                # matmul stationary operand has a single free dim
                for g in range(G):
                    for c in range(HC):
                        nc.vector.tensor_copy(
                            Ag[:, (g * HC + c) * 128 : (g * HC + c) * 128 + 128],
                            Asc_sb[:, c * 512 : (c + 1) * 512].rearrange(
                                "p (k n) -> p k n", k=16
                            )[:, :, GS * g : GS * (g + 1)],
                        )
                for blk in range(8):
                    Wab = php.tile([128, HC * 512], BF16, tag="Wab")
                    nc.sync.dma_start(
                        Wab[:].rearrange("p (c x) -> p c x", c=HC),
                        ap_Wattn.rearrange("(c p) x -> p c x", p=128)[
                            :, :, 512 * blk : 512 * (blk + 1)
                        ],
                    )
                    for g in range(G):
                        psp = psP.tile([128, 512], F32, tag="psp")
                        for c in range(HC):
                            nc.tensor.matmul(
                                psp[:],
                                Ag[:, (g * HC + c) * 128 : (g * HC + c) * 128 + 128],
                                Wab[:, c * 512 : (c + 1) * 512],
                                start=(c == 0),
                                stop=(c == HC - 1),
                            )
                        nc.vector.tensor_copy(
                            P_sb[:, g * H4 + 512 * blk : g * H4 + 512 * (blk + 1)],
                            psp[:],
                        )

            # ---------------------- phase C: recurrence ----------------------
            with tc.tile_pool(name="wrk", bufs=2) as wrk, \
                 tc.tile_pool(name="psc", bufs=1, space="PSUM") as psc_pool, \
                 tc.tile_pool(name="pwx", bufs=1, space="PSUM") as pwx_pool, \
                 tc.tile_pool(name="pstr", bufs=1, space="PSUM") as pstr_pool, \
                 tc.tile_pool(name="paT", bufs=1, space="PSUM") as paT_pool:
                q4 = lambda ap: ap.rearrange("p (q c) -> p q c", q=4)
                chunks = [
                    (W, t4)
                    for t4 in (1, 2, 3)
                    if 16 * t4 < t_steps
                    for W in range(32)
                ]
                ci = 0
                for t in range(t_steps):
                    # prefetched xproj slices for this step
                    xptf = wrk.tile([128, 1024], BF16, tag="xpt", name=f"xpt_{t}")
                    xpt = [xptf[:, 512 * r : 512 * (r + 1)] for r in range(2)]
                    for r in range(2):
                        nc.sync.dma_start(
                            xpt[r].rearrange("p (c n) -> p c n", c=16),
                            xps[r, :, :, t].rearrange("q j p n -> p (q j) n"),
                        )

                    # -- scores: per-group (8-sample) products against Ag with
                    # 4-way col tiling, group-local diag extract, softmax
                    pscg = psc_pool.tile([128, 128], F32, tag="psc")
                    for c in range(HC):
                        for g in range(G):
                            nc.tensor.matmul(
                                pscg[32 * g : 32 * (g + 1), :],
                                uTh[:, c * 32 : (c + 1) * 32],
                                Ag[:, (g * HC + c) * 128 : (g * HC + c + 1) * 128],
                                start=(c == 0),
                                stop=(c == HC - 1),
                                skip_group_check=True,
                                tile_position=(0, 32 * g),
                            )
                    scm = wrk.tile([128, 128], F32, tag="scm")
                    nc.vector.tensor_mul(scm[:], pscg[:], Mdiag8[:])
                    scores = wrk.tile([128, 16], F32, tag="scores")
                    nc.vector.tensor_reduce(
                        scores[:],
                        scm[:].rearrange("p (k n) -> p k n", k=16),
                        axis=AX.X,
                        op=ALU.add,
                    )
                    nmx = wrk.tile([128, 1], F32, tag="nmx")
                    nc.vector.tensor_reduce(
                        nmx[:], scores[:], axis=AX.X, op=ALU.max, negate=True
                    )
                    # softmax via the sigmoid table (keeps every ACT op in the
                    # sigmoid_and_others set -> one table load for the kernel):
                    # y = sigmoid(s - m) in (0, 0.5], e^(s-m) = y / (1 - y)
                    ysig = wrk.tile([128, 16], F32, tag="ysig")
                    nc.scalar.activation(
                        ysig[:], scores[:], AF.Sigmoid, bias=nmx[:], scale=1.0
                    )
                    omy = wrk.tile([128, 16], F32, tag="omy")
                    nc.vector.tensor_scalar(
                        omy[:], ysig[:], -1.0, 1.0, ALU.mult, ALU.add
                    )
                    romy = wrk.tile([128, 16], F32, tag="romy")
                    nc.vector.reciprocal(romy[:], omy[:])
                    ex = wrk.tile([128, 16], F32, tag="ex")
                    esum = wrk.tile([128, 1], F32, tag="esum")
                    nc.vector.scalar_tensor_tensor(
                        ex[:], ysig[:], 1.0, romy[:], ALU.mult, ALU.mult,
                        accum_out=esum[:],
                    )
                    rcp = wrk.tile([128, 1], F32, tag="rcp")
                    nc.vector.reciprocal(rcp[:], esum[:])
                    w2 = wrk.tile([128, 16], BF16, tag="w2")
                    nc.vector.tensor_scalar_mul(w2[:], ex[:], rcp[:])
                    # compact the (g, m)-partition w to wT[k, n32] on PE
                    wTps = pwx_pool.tile([16, 32], F32, tag="wTps")
                    nc.tensor.matmul(wTps[:], w2[:], selT[:], start=True, stop=True)
                    wT = wrk.tile([16, 32], BF16, tag="wT")
                    nc.vector.tensor_copy(wT[:], wTps[:])
                    wrep = wrk.tile([16, 128], BF16, tag="wrep")
                    for rep in range(4):
                        nc.vector.tensor_copy(
                            wrep[:, 32 * rep : 32 * (rep + 1)], wT[:]
                        )
                    pwx = pwx_pool.tile([128, 128], F32, tag="pwx")
                    nc.tensor.matmul(pwx[:], E16[:], wrep[:], start=True, stop=True)
                    masked = wrk.tile([128, 512], BF16, tag="masked")
                    for g in range(G):
                        nc.vector.tensor_mul(
                            masked[:, g * 128 : (g + 1) * 128],
                            pwx[:],
                            M32R[:, g * 128 : (g + 1) * 128],
                        )

                    # -- gates: h @ Wh + sum_k w_k P_k into 2 column-tiled strips
                    strips = [
                        pstr_pool.tile([128, 512], F32, tag=f"strip{r}",
                                       name=f"strip{r}_{t}")
                        for r in range(2)
                    ]
                    for c in range(HC):
                        for r in range(2):
                            for j in range(4):
                                nc.tensor.matmul(
                                    strips[r][32 * j : 32 * (j + 1), :],
                                    uTh[:, c * 32 : (c + 1) * 32],
                                    Wh_sb[:, c * H4 + j * 1024 + r * 512 : c * H4 + j * 1024 + r * 512 + 512],
                                    start=(c == 0),
                                    stop=False,
                                    skip_group_check=True,
                                    tile_position=(0, 32 * j),
                                )
                    for r in range(2):
                        for g in range(G):
                            for j in range(4):
                                nc.tensor.matmul(
                                    strips[r][32 * j : 32 * (j + 1), :],
                                    masked[:, g * 128 + 32 * j : g * 128 + 32 * (j + 1)],
                                    P_sb[:, g * H4 + j * 1024 + r * 512 : g * H4 + j * 1024 + r * 512 + 512],
                                    start=False,
                                    stop=(g == G - 1),
                                    skip_group_check=True,
                                    tile_position=(0, 32 * j),
                                )

                    # -- transpose strips, add xproj, activations, cell update
                    for r in range(2):
                        sg = wrk.tile([128, 512], BF16, tag=f"sg{r}")
                        nc.scalar.copy(sg[:], strips[r][:])
                        pat = paT_pool.tile(
                            [128, 512], BF16, tag="pat", name=f"pat{r}_{t}"
                        )
                        for q in range(4):
                            nc.tensor.matmul(
                                pat[:, 128 * q : 128 * (q + 1)],
                                sg[:, 128 * q : 128 * (q + 1)],
                                eye[:],
                                is_transpose=True,
                                start=(q == 0),
                                stop=(q == 3),
                            )
                        ssum = wrk.tile([128, 512], BF16, tag=f"ssum{r}")
                        nc.vector.tensor_add(ssum[:], pat[:], xpt[r])
                        act = wrk.tile([128, 512], F32, tag=f"act{r}")
                        nc.scalar.activation(
                            q4(act[:])[:, :, 0:96], q4(ssum[:])[:, :, 0:96], AF.Sigmoid
                        )
                        nc.scalar.activation(
                            q4(act[:])[:, :, 96:128], q4(ssum[:])[:, :, 96:128], AF.Tanh
                        )
                        i_v = q4(act[:])[:, :, 0:32]
                        f_v = q4(act[:])[:, :, 32:64]
                        o_v = q4(act[:])[:, :, 64:96]
                        g_v = q4(act[:])[:, :, 96:128]
                        cview = cT[:, 128 * r : 128 * (r + 1)].rearrange(
                            "p (q n) -> p q n", q=4
                        )
                        ig = wrk.tile([128, 128], F32, tag=f"ig{r}")
                        nc.vector.tensor_mul(q4(ig[:]), i_v, g_v)
                        fc = wrk.tile([128, 128], F32, tag=f"fc{r}")
                        nc.vector.tensor_mul(q4(fc[:]), f_v, cview)
                        nc.vector.tensor_add(
                            cview, q4(ig[:]), q4(fc[:])
                        )
                        tch = wrk.tile([128, 128], F32, tag=f"tch{r}")
                        nc.scalar.activation(
                            tch[:], cT[:, 128 * r : 128 * (r + 1)], AF.Tanh
                        )
                        h32 = wrk.tile([128, 128], F32, tag=f"h32{r}")
                        nc.vector.tensor_mul(
                            h32[:].rearrange("p (q n) -> p q n", q=4),
                            o_v,
                            tch[:].rearrange("p (q n) -> p q n", q=4),
                        )
                        # write h into uTh (bf16) for step t+1
                        nc.vector.tensor_copy(
                            uTh[:, 128 * r : 128 * (r + 1)], h32[:]
                        )
                        nc.sync.dma_start(
                            outT[t, 512 * r : 512 * (r + 1), :].rearrange(
                                "(q p) n -> p q n", p=128
                            ),
                            h32[:].rearrange("p (q n) -> p q n", q=4),
                        )

                    # interleave deferred xproj chunks (t4 >= 1) into the
                    # recurrence to fill PE-idle gaps and keep HAM warm
                    want = min(len(chunks), int((t + 1) * 2.4) + 1)
                    while ci < want:
                        xproj_chunk(*chunks[ci])
                        ci += 1
                while ci < len(chunks):
                    xproj_chunk(*chunks[ci])
                    ci += 1
            _ax.close()
    nc.compile()
    return nc


def _prep_shards(inputs):
    x = np.asarray(inputs["x"], np.float32)
    A = np.asarray(inputs["A"], np.float32)
    Wx = np.asarray(inputs["Wx"], np.float32)
    Wh = np.asarray(inputs["Wh"], np.float32)
    Wattn = np.asarray(inputs["Wattn"], np.float32)
    b = np.asarray(inputs["b"], np.float32)

    Wx_bf = np.ascontiguousarray(Wx.astype(BF))
    Wh_bf = np.ascontiguousarray(Wh.astype(BF))
    Wa_bf = np.ascontiguousarray(Wattn.astype(BF))
    bT = np.ascontiguousarray(b.reshape(32, 128).T.astype(np.float32))

    in_maps = []
    for i in range(NCORES):
        ns = slice(NL * i, NL * (i + 1))
        xT = x[ns].transpose(2, 1, 0).reshape(D, T * NL)
        Asc = A[ns].reshape(NL, H, 16).transpose(1, 2, 0).reshape(H, 512)
        in_maps.append(
            {
                "xT": np.ascontiguousarray(xT.astype(BF)),
                "Asc": np.ascontiguousarray(Asc.astype(BF)),
                "Asc32": np.ascontiguousarray(Asc.astype(np.float32)),
                "Wx": Wx_bf,
                "Wh": Wh_bf,
                "Wattn": Wa_bf,
                "bT": bT,
            }
        )
    return in_maps


def _get_nc():
    global _built
    if _built is None:
        _built = _build_nc()
    return _built


def _run(inputs, **kwargs):
    nc = _get_nc()
    in_maps = _prep_shards(inputs)
    res = bass_utils.run_bass_kernel_spmd(
        nc, in_maps, core_ids=list(range(NCORES)), **kwargs
    )
    out = np.empty((N, T, H), np.float32)
    for i in range(NCORES):
        out[NL * i : NL * (i + 1)] = res.results[i]["outT"].transpose(2, 0, 1)
    return out, res


def kernel(**inputs):
    out, _ = _run(inputs)
    return out



# revision 50
# speedup vs baseline: 1.4560x; 1.0056x over previous
"""Trainium2 Bass kernel for nn_CaptioningRNN (attention LSTM over T=64).

Data-parallel over the batch: N=256 samples split across 8 NeuronCores
(32 samples/core), weights replicated, no collectives.

Per-core algorithm (all matmuls bf16 on the TensorEngine, state in f32):
  1. xproj phase: xpT = (x @ Wx + b) computed transposed via Wx-stationary
     matmuls, stored to a DRAM scratch laid out so the per-step slice loads
     as a clean [128, 512] tile.
  2. P phase: P[n, k, :] = A[n, :, k] @ Wattn precomputed once (the
     attention context contribution to the gates becomes a w-weighted sum
     of P rows, replacing a per-step [32,1024]@[1024,4096] matmul).
     h0 = c0 = mean_k(A) computed on device from an f32 copy of A.
  3. Recurrence (64 steps):
     - scores via hT-chunk matmuls against a permuted A (cross-sample
       products in PSUM, diagonal extracted with a mask+reduce on DVE)
     - softmax on [32,16] (ACT exp with fused row-sum)
     - w transposed (DVE 32x32 stream transpose) and expanded to the
       (k, n_g)-partition block-diagonal layout via a one-hot matmul + mask
     - gates = h @ Wh + sum_k w_k P_k accumulated into two PSUM strips
       using 4-way tensor-engine column tiling (beats the M=32 small-batch
       penalty ~4x)
     - strips transposed on PE; cell math done in h-on-partition space so
       i/f/o/g land on identical lanes (no cross-partition ops needed)
  4. Output written transposed [t, h, n]; host reassembles to (N, T, H).
"""

from contextlib import ExitStack

import numpy as np
import ml_dtypes

import concourse.bacc as bacc
import concourse.mybir as mybir
from concourse import bass_utils
from concourse.tile import TileContext

F32, BF16 = mybir.dt.float32, mybir.dt.bfloat16
AF = mybir.ActivationFunctionType
ALU = mybir.AluOpType
AX = mybir.AxisListType
BF = ml_dtypes.bfloat16

N, T, D, H = 256, 64, 1024, 1024
NCORES = 8
NL = N // NCORES          # 32 samples per core
HC = 8                    # 128-row chunks of D/H
G, GS = 4, 8              # sample groups of 8 (for the (k, n_g) 128-partition layout)
H4 = 4 * H                # 4096 gate columns

_built = None


def _consts():
    # E16[k', 8k + n] = (k' == k): one-hot expansion of wT rows onto the
    # (k-major, n_g-minor) 128-partition layout.
    e16 = np.zeros((16, 128), dtype=BF)
    for k in range(16):
        e16[k, 8 * k : 8 * k + 8] = 1
    # M32R[p, 128 g + 32 rep + m] = (m % 8 == p % 8) & (m // 8 == g):
    # block-diagonal mask producing masked_g = w[m, k(p)] only for group-g
    # samples, replicated 4x for the column-tiled matmuls.
    p = np.arange(128)[:, None]
    m = np.arange(32)[None, :]
    m32r = np.zeros((128, 512), dtype=BF)
    for g in range(4):
        blk = ((m % 8 == p % 8) & (m // 8 == g)).astype(BF)
        for rep in range(4):
            m32r[:, 128 * g + 32 * rep : 128 * g + 32 * rep + 32] = blk
    # Mdiag8[32 g + m, 8 k + n] = (m == 8 g + n) / 32: extracts the
    # group-local diagonal of the score products (stationary = all 32
    # samples, moving = group-g A columns) and applies the 1/sqrt(H) scale.
    md8 = np.zeros((128, 128), dtype=np.float32)
    for g in range(4):
        for n in range(8):
            for k in range(16):
                md8[32 * g + 8 * g + n, 8 * k + n] = 1.0 / 32.0
    # selT[32 g + (8 g + n), 8 g + n] = 1: compacts the block-diagonal w
    # layout to wT[k, n] via a single PE matmul (stationary = w2).
    sel = np.zeros((128, 32), dtype=BF)
    for g in range(4):
        for n in range(8):
            sel[32 * g + 8 * g + n, 8 * g + n] = 1
    return e16, m32r, md8, sel


def _build_nc(t_steps=T):
    nc = bacc.Bacc(trn_type="TRN2", target_bir_lowering=False, debug=False)

    ap_xT = nc.dram_tensor("xT", [D, T * NL], BF16, kind="ExternalInput").ap()
    ap_Asc = nc.dram_tensor("Asc", [H, 512], BF16, kind="ExternalInput").ap()
    ap_Asc32 = nc.dram_tensor("Asc32", [H, 512], F32, kind="ExternalInput").ap()
    ap_Wx = nc.dram_tensor("Wx", [D, H4], BF16, kind="ExternalInput").ap()
    ap_Wh = nc.dram_tensor("Wh", [H, H4], BF16, kind="ExternalInput").ap()
    ap_Wattn = nc.dram_tensor("Wattn", [H, H4], BF16, kind="ExternalInput").ap()
    ap_bT = nc.dram_tensor("bT", [128, 32], F32, kind="ExternalInput").ap()
    outT = nc.dram_tensor("outT", [T, H, NL], F32, kind="ExternalOutput").ap()
    # xps[r, q, j, t, p, n] = xproj[t][n, j*1024 + r*512 + q*128 + p]
    # ((q, j) outermost so phase-A stores and per-step loads are both
    # contiguous 8 KiB (p, n) blocks per (q, j))
    xps = nc.dram_tensor("xps", [2, 4, 4, T, 128, NL], BF16, kind="Internal").ap()

    e16_np, m32r_np, md8_np, sel_np = _consts()
    eye_d = nc.inline_tensor(np.eye(128, dtype=BF), "c_eye")
    e16_d = nc.inline_tensor(e16_np, "c_e16")
    m32r_d = nc.inline_tensor(m32r_np, "c_m32r")
    md8_d = nc.inline_tensor(md8_np, "c_mdiag8")
    sel_d = nc.inline_tensor(sel_np, "c_selT")

    with TileContext(nc) as tc:
        with tc.tile_pool(name="pers", bufs=1) as pers:
            Wh_sb = pers.tile([128, HC * H4], BF16, tag="Wh")
            Asc_sb = pers.tile([128, HC * 512], BF16, tag="Asc")
            P_sb = pers.tile([128, G * H4], BF16, tag="P")
            uTh = pers.tile([128, HC * 32], BF16, tag="uTh")
            cT = pers.tile([128, 256], F32, tag="cT")
            eye = pers.tile([128, 128], BF16, tag="eye")
            E16 = pers.tile([16, 128], BF16, tag="E16")
            M32R = pers.tile([128, 512], BF16, tag="M32R")
            Mdiag8 = pers.tile([128, 128], F32, tag="Mdiag8")
            selT = pers.tile([128, 32], BF16, tag="selT")
            b_sb = pers.tile([128, 32], F32, tag="bT")
            Ag = pers.tile([128, G * HC * 128], BF16, tag="Ag")

            nc.sync.dma_start(eye[:], eye_d.ap()[:])
            nc.sync.dma_start(E16[:], e16_d.ap()[:])
            nc.sync.dma_start(M32R[:], m32r_d.ap()[:])
            nc.sync.dma_start(Mdiag8[:], md8_d.ap()[:])
            nc.sync.dma_start(selT[:], sel_d.ap()[:])
            nc.sync.dma_start(b_sb[:], ap_bT[:])
            nc.sync.dma_start(
                Wh_sb[:].rearrange("p (c x) -> p c x", c=HC),
                ap_Wh.rearrange("(c p) x -> p c x", p=128),
            )
            nc.sync.dma_start(
                Asc_sb[:].rearrange("p (c x) -> p c x", c=HC),
                ap_Asc.rearrange("(c p) x -> p c x", p=128),
            )

            # ---------------- phase A: xproj -> DRAM scratch ----------------
            # Pools stay open through the recurrence so t4>=1 chunks can be
            # interleaved between steps (fills PE-idle gaps, keeps HAM warm).
            _ax = ExitStack()
            phx1 = _ax.enter_context(tc.tile_pool(name="phx1", bufs=1))
            phx = _ax.enter_context(tc.tile_pool(name="phx", bufs=3))
            psX = _ax.enter_context(tc.tile_pool(name="psX", bufs=2, space="PSUM"))
            xT_sb = phx1.tile([128, HC * T * NL], BF16, tag="xTsb")
            nc.sync.dma_start(
                xT_sb[:].rearrange("p (c x) -> p c x", c=HC),
                ap_xT.rearrange("(c p) x -> p c x", p=128),
            )

            def xproj_chunk(W, t4):
                j, r, q = W // 8, (W % 8) // 4, W % 4
                Wxb = phx.tile(
                    [128, HC * 128], BF16, tag="Wxb", name=f"Wxb_{W}_{t4}"
                )
                nc.sync.dma_start(
                    Wxb[:].rearrange("p (c x) -> p c x", c=HC),
                    ap_Wx.rearrange("(c p) x -> p c x", p=128)[
                        :, :, 128 * W : 128 * (W + 1)
                    ],
                )
                psx = psX.tile([128, 512], F32, tag="psx", name=f"psx_{W}_{t4}")
                for c in range(HC):
                    nc.tensor.matmul(
                        psx[:],
                        Wxb[:, c * 128 : (c + 1) * 128],
                        xT_sb[:, c * T * NL + 512 * t4 : c * T * NL + 512 * (t4 + 1)],
                        start=(c == 0),
                        stop=(c == HC - 1),
                    )
                sxp = phx.tile([128, 512], BF16, tag="sxp", name=f"sxp_{W}_{t4}")
                nc.scalar.add(sxp[:], psx[:], b_sb[:, W : W + 1])
                nc.sync.dma_start(
                    xps[r, q, j, 16 * t4 : 16 * (t4 + 1)].transpose([1, 0, 2]),
                    sxp[:].rearrange("p (t n) -> p t n", t=16),
                )

            for W in range(32):
                xproj_chunk(W, 0)

            # ------------- phase B: P precompute + h0/c0 init -------------
            with tc.tile_pool(name="php1", bufs=1) as php1, \
                 tc.tile_pool(name="php", bufs=3) as php, \
                 tc.tile_pool(name="psP", bufs=2, space="PSUM") as psP:
                A32 = php1.tile([128, HC * 512], F32, tag="A32")
                nc.sync.dma_start(
                    A32[:].rearrange("p (c x) -> p c x", c=HC),
                    ap_Asc32.rearrange("(c p) x -> p c x", p=128),
                )
                for c in range(HC):
                    h0s = php.tile([128, 32], F32, tag="h0s")
                    nc.vector.tensor_reduce(
                        h0s[:],
                        A32[:, c * 512 : (c + 1) * 512].rearrange(
                            "p (k n) -> p n k", k=16
                        ),
                        axis=AX.X,
                        op=ALU.add,
                    )
                    nc.vector.tensor_scalar_mul(
                        cT[:, 32 * c : 32 * (c + 1)], h0s[:], 1.0 / 16.0
                    )
                    nc.vector.tensor_copy(
                        uTh[:, 32 * c : 32 * (c + 1)],
                        cT[:, 32 * c : 32 * (c + 1)],
                    )
                # contiguous staging of the group-selected A columns so the
                # matmul stationary operand has a single free dim
                for g in range(G):
                    for c in range(HC):
                        nc.vector.tensor_copy(
                            Ag[:, (g * HC + c) * 128 : (g * HC + c) * 128 + 128],
                            Asc_sb[:, c * 512 : (c + 1) * 512].rearrange(
                                "p (k n) -> p k n", k=16
                            )[:, :, GS * g : GS * (g + 1)],
                        )
                for blk in range(8):
                    Wab = php.tile([128, HC * 512], BF16, tag="Wab")
                    nc.sync.dma_start(
                        Wab[:].rearrange("p (c x) -> p c x", c=HC),
                        ap_Wattn.rearrange("(c p) x -> p c x", p=128)[
                            :, :, 512 * blk : 512 * (blk + 1)
                        ],
                    )
                    for g in range(G):
                        psp = psP.tile([128, 512], F32, tag="psp")
                        for c in range(HC):
                            nc.tensor.matmul(
                                psp[:],
                                Ag[:, (g * HC + c) * 128 : (g * HC + c) * 128 + 128],
                                Wab[:, c * 512 : (c + 1) * 512],
                                start=(c == 0),
                                stop=(c == HC - 1),
                            )
                        nc.vector.tensor_copy(
                            P_sb[:, g * H4 + 512 * blk : g * H4 + 512 * (blk + 1)],
                            psp[:],
                        )

            # ---------------------- phase C: recurrence ----------------------
            with tc.tile_pool(name="wrk", bufs=2) as wrk, \
                 tc.tile_pool(name="psc", bufs=1, space="PSUM") as psc_pool, \
                 tc.tile_pool(name="pwx", bufs=1, space="PSUM") as pwx_pool, \
                 tc.tile_pool(name="pstr", bufs=1, space="PSUM") as pstr_pool, \
                 tc.tile_pool(name="paT", bufs=1, space="PSUM") as paT_pool:
                q4 = lambda ap: ap.rearrange("p (q c) -> p q c", q=4)
                chunks = [
                    (W, t4)
                    for t4 in (1, 2, 3)
                    if 16 * t4 < t_steps
                    for W in range(32)
                ]
                ci = 0
                for t in range(t_steps):
                    # prefetched xproj slices for this step
                    xptf = wrk.tile([128, 1024], BF16, tag="xpt", name=f"xpt_{t}")
                    xpt = [xptf[:, 512 * r : 512 * (r + 1)] for r in range(2)]
                    for r in range(2):
                        nc.gpsimd.dma_start(
                            xpt[r].rearrange("p (c n) -> p c n", c=16),
                            xps[r, :, :, t].rearrange("q j p n -> p (q j) n"),
                        )

                    # -- scores: per-group (8-sample) products against Ag with
                    # 4-way col tiling, group-local diag extract, softmax
                    pscg = psc_pool.tile([128, 128], F32, tag="psc")
                    for c in range(HC):
                        for g in range(G):
                            nc.tensor.matmul(
                                pscg[32 * g : 32 * (g + 1), :],
                                uTh[:, c * 32 : (c + 1) * 32],
                                Ag[:, (g * HC + c) * 128 : (g * HC + c + 1) * 128],
                                start=(c == 0),
                                stop=(c == HC - 1),
                                skip_group_check=True,
                                tile_position=(0, 32 * g),
                            )
                    scm = wrk.tile([128, 128], F32, tag="scm")
                    nc.vector.tensor_mul(scm[:], pscg[:], Mdiag8[:])
                    scores = wrk.tile([128, 16], F32, tag="scores")
                    nc.vector.tensor_reduce(
                        scores[:],
                        scm[:].rearrange("p (k n) -> p k n", k=16),
                        axis=AX.X,
                        op=ALU.add,
                    )
                    nmx = wrk.tile([128, 1], F32, tag="nmx")
                    nc.vector.tensor_reduce(
                        nmx[:], scores[:], axis=AX.X, op=ALU.max, negate=True
                    )
                    # softmax via the sigmoid table (keeps every ACT op in the
                    # sigmoid_and_others set -> one table load for the kernel):
                    # y = sigmoid(s - m) in (0, 0.5], e^(s-m) = y / (1 - y)
                    ysig = wrk.tile([128, 16], F32, tag="ysig")
                    nc.scalar.activation(
                        ysig[:], scores[:], AF.Sigmoid, bias=nmx[:], scale=1.0
                    )
                    omy = wrk.tile([128, 16], F32, tag="omy")
                    nc.vector.tensor_scalar(
                        omy[:], ysig[:], -1.0, 1.0, ALU.mult, ALU.add
                    )
                    romy = wrk.tile([128, 16], F32, tag="romy")
                    nc.vector.reciprocal(romy[:], omy[:])
                    ex = wrk.tile([128, 16], F32, tag="ex")
                    esum = wrk.tile([128, 1], F32, tag="esum")
                    nc.vector.scalar_tensor_tensor(
                        ex[:], ysig[:], 1.0, romy[:], ALU.mult, ALU.mult,
                        accum_out=esum[:],
                    )
                    rcp = wrk.tile([128, 1], F32, tag="rcp")
                    nc.vector.reciprocal(rcp[:], esum[:])
                    w2 = wrk.tile([128, 16], BF16, tag="w2")
                    nc.vector.tensor_scalar_mul(w2[:], ex[:], rcp[:])
                    # compact the (g, m)-partition w to wT[k, n32] on PE
                    wTps = pwx_pool.tile([16, 32], F32, tag="wTps")
                    nc.tensor.matmul(wTps[:], w2[:], selT[:], start=True, stop=True)
                    wT = wrk.tile([16, 32], BF16, tag="wT")
                    nc.vector.tensor_copy(wT[:], wTps[:])
                    # expand w onto the (k, n8)-partition block layout: one
                    # matmul with a stride-0 16x-repeated moving operand, then
                    # a single masked multiply
                    pwx = pwx_pool.tile([128, 512], F32, tag="pwx")
                    nc.tensor.matmul(
                        pwx[:],
                        E16[:],
                        wT[:].unsqueeze(1).broadcast_to([16, 16, 32]),
                        start=True,
                        stop=True,
                    )
                    masked = wrk.tile([128, 512], BF16, tag="masked")
                    nc.vector.tensor_mul(masked[:], pwx[:], M32R[:])

                    # -- gates: h @ Wh + sum_k w_k P_k into 2 column-tiled strips
                    strips = [
                        pstr_pool.tile([128, 512], F32, tag=f"strip{r}",
                                       name=f"strip{r}_{t}")
                        for r in range(2)
                    ]
                    for c in range(HC):
                        for r in range(2):
                            for j in range(4):
                                nc.tensor.matmul(
                                    strips[r][32 * j : 32 * (j + 1), :],
                                    uTh[:, c * 32 : (c + 1) * 32],
                                    Wh_sb[:, c * H4 + j * 1024 + r * 512 : c * H4 + j * 1024 + r * 512 + 512],
                                    start=(c == 0),
                                    stop=False,
                                    skip_group_check=True,
                                    tile_position=(0, 32 * j),
                                )
                    for r in range(2):
                        for g in range(G):
                            for j in range(4):
                                nc.tensor.matmul(
                                    strips[r][32 * j : 32 * (j + 1), :],
                                    masked[:, g * 128 + 32 * j : g * 128 + 32 * (j + 1)],
                                    P_sb[:, g * H4 + j * 1024 + r * 512 : g * H4 + j * 1024 + r * 512 + 512],
                                    start=False,
                                    stop=(g == G - 1),
                                    skip_group_check=True,
                                    tile_position=(0, 32 * j),
                                )

                    # -- transpose strips, add xproj, activations, cell update
                    h32all = wrk.tile([128, 256], F32, tag="h32", name=f"h32_{t}")
                    for r in range(2):
                        sg = wrk.tile([128, 512], BF16, tag=f"sg{r}")
                        nc.scalar.copy(sg[:], strips[r][:])
                        pat = paT_pool.tile(
                            [128, 512], BF16, tag="pat", name=f"pat{r}_{t}"
                        )
                        for q in range(4):
                            nc.tensor.matmul(
                                pat[:, 128 * q : 128 * (q + 1)],
                                sg[:, 128 * q : 128 * (q + 1)],
                                eye[:],
                                is_transpose=True,
                                start=(q == 0),
                                stop=(q == 3),
                            )
                        ssum = wrk.tile([128, 512], BF16, tag=f"ssum{r}")
                        nc.vector.tensor_add(ssum[:], pat[:], xpt[r])
                        act = wrk.tile([128, 512], F32, tag=f"act{r}")
                        nc.scalar.activation(
                            q4(act[:])[:, :, 0:96], q4(ssum[:])[:, :, 0:96], AF.Sigmoid
                        )
                        nc.scalar.activation(
                            q4(act[:])[:, :, 96:128], q4(ssum[:])[:, :, 96:128], AF.Tanh
                        )
                        i_v = q4(act[:])[:, :, 0:32]
                        f_v = q4(act[:])[:, :, 32:64]
                        o_v = q4(act[:])[:, :, 64:96]
                        g_v = q4(act[:])[:, :, 96:128]
                        cview = cT[:, 128 * r : 128 * (r + 1)].rearrange(
                            "p (q n) -> p q n", q=4
                        )
                        ig = wrk.tile([128, 128], F32, tag=f"ig{r}")
                        nc.vector.tensor_mul(q4(ig[:]), i_v, g_v)
                        fc = wrk.tile([128, 128], F32, tag=f"fc{r}")
                        nc.vector.tensor_mul(q4(fc[:]), f_v, cview)
                        nc.vector.tensor_add(
                            cview, q4(ig[:]), q4(fc[:])
                        )
                        tch = wrk.tile([128, 128], F32, tag=f"tch{r}")
                        nc.scalar.activation(
                            tch[:], cT[:, 128 * r : 128 * (r + 1)], AF.Tanh
                        )
                        h32 = h32all[:, 128 * r : 128 * (r + 1)]
                        nc.vector.tensor_mul(
                            h32.rearrange("p (q n) -> p q n", q=4),
                            o_v,
                            tch[:].rearrange("p (q n) -> p q n", q=4),
                        )
                        # write h into uTh (bf16) for step t+1
                        nc.vector.tensor_copy(
                            uTh[:, 128 * r : 128 * (r + 1)], h32
                        )
                    nc.gpsimd.dma_start(
                        outT[t].rearrange("(r q p) n -> p r q n", r=2, p=128),
                        h32all[:].rearrange("p (r q n) -> p r q n", r=2, q=4),
                    )

                    # interleave deferred xproj chunks (t4 >= 1) into the
                    # recurrence to fill PE-idle gaps and keep HAM warm
                    want = min(len(chunks), int((t + 1) * 2.4) + 1)
                    while ci < want:
                        xproj_chunk(*chunks[ci])
                        ci += 1
                while ci < len(chunks):
                    xproj_chunk(*chunks[ci])
                    ci += 1
            _ax.close()
    nc.compile()
    return nc


def _prep_shards(inputs):
    x = np.asarray(inputs["x"], np.float32)
    A = np.asarray(inputs["A"], np.float32)
    Wx = np.asarray(inputs["Wx"], np.float32)
    Wh = np.asarray(inputs["Wh"], np.float32)
    Wattn = np.asarray(inputs["Wattn"], np.float32)
    b = np.asarray(inputs["b"], np.float32)

    Wx_bf = np.ascontiguousarray(Wx.astype(BF))
    Wh_bf = np.ascontiguousarray(Wh.astype(BF))
    Wa_bf = np.ascontiguousarray(Wattn.astype(BF))
    bT = np.ascontiguousarray(b.reshape(32, 128).T.astype(np.float32))

    in_maps = []
    for i in range(NCORES):
        ns = slice(NL * i, NL * (i + 1))
        xT = x[ns].transpose(2, 1, 0).reshape(D, T * NL)
        Asc = A[ns].reshape(NL, H, 16).transpose(1, 2, 0).reshape(H, 512)
        in_maps.append(
            {
                "xT": np.ascontiguousarray(xT.astype(BF)),
                "Asc": np.ascontiguousarray(Asc.astype(BF)),
                "Asc32": np.ascontiguousarray(Asc.astype(np.float32)),
                "Wx": Wx_bf,
                "Wh": Wh_bf,
                "Wattn": Wa_bf,
                "bT": bT,
            }
        )
    return in_maps


def _get_nc():
    global _built
    if _built is None:
        _built = _build_nc()
    return _built


def _run(inputs, **kwargs):
    nc = _get_nc()
    in_maps = _prep_shards(inputs)
    res = bass_utils.run_bass_kernel_spmd(
        nc, in_maps, core_ids=list(range(NCORES)), **kwargs
    )
    out = np.empty((N, T, H), np.float32)
    for i in range(NCORES):
        out[NL * i : NL * (i + 1)] = res.results[i]["outT"].transpose(2, 0, 1)
    return out, res


def kernel(**inputs):
    out, _ = _run(inputs)
    return out

